# revision 41
# baseline (speedup 1.0000x reference)
"""Trainium2 Bass kernel for nn_EventSampler (thinning / rejection sampling).

kernel(**inputs) takes FULL unsharded inputs, shards batch across 8 cores
(2 batches = 256 (b,l) pairs per core), runs one SPMD Bass program, returns
the full output.

v3 structure (cost-model driven; per core):
  host prep: e-axis of (exp_raw, unif_numbers) sorted ascending by exp_raw
  per (b,l) pair, u transposed to [E, S, P] so the accept test runs in an
  e-on-partition layout.

  phase 1 (pair-layout, f32, same operation classes as the validated
  baseline): 25-point grid (20 bound-scan + 5 Chebyshev-Lobatto nodes) ->
  bounds -> b15/invb -> degree-4 polynomial tot(x) at the sorted draws ->
  q = tot*invb.  q is transposed to e-layout via PE transpose.

  phase 2 (e-layout, 16 pipelined pieces = 2 e-chunks x 8 4-row s-slices):
    m = (u < q) on DVE (the only full-size f32 pass),
    first-accept extraction on PE: counts = (200*I - 200*strict_tri) @ m
    (+ cross-chunk -200*ones @ m0; 512-col matmul halves, bank-aligned so
    start=True resets don't clobber the sibling half),
    IND = Act(Exp, counts - 200) in {0,1} exactly,
    fa = IND * raw_sorted (bf16; rows split 3 DVE / 1 GpSimd),
    val = ones @ fa = raw* (or exact 0 if no accept), accumulated over both
    e-chunks in PSUM, parked to SBUF via Act copy.
  val rows return to pair-layout via PE transpose; tail (who = val>0,
  acc = val*invb, fallback max(last_raw*invb, dtb), min 1e5) is tiny.

Decision-critical arithmetic stays f32; bf16 only on the value path
(res tolerance 2e-2; host-sim on the real inputs shows max rel 3.9e-3).
Cost-model timeline: 62384 ns vs 80379 ns for the previous kernel.
"""

import os
import sys

import numpy as np

for _p in ("/opt/trn_rl_repo",):
    if _p not in sys.path and os.path.isdir(_p):
        sys.path.insert(0, _p)

import concourse.bacc as bacc
import concourse.tile as tile
import concourse.mybir as mybir
from concourse.bass_utils import run_bass_kernel_spmd

F32 = mybir.dt.float32
BF16 = mybir.dt.bfloat16

B, L, M = 16, 128, 32
S, E, S0 = 32, 256, 20
OVER = 1.5
KC = 5
G = S0 + KC
N_CORES = 8
BPC = B // N_CORES
P = BPC * L                     # pairs per core = 256
NP = 128
NCH = P // NP                   # pair chunks = 2
ECH = E // NP                   # e chunks = 2
RS = 4                          # s-rows per phase-2 piece
NSP = S // RS                   # s-pieces = 8
NPC = RS * P                    # free cols per piece = 1024
BIG = 200.0

# pk layout: aemb | tds | ddom | fourd | dtb | lastraw
WPK = M + 5
# cst (f32): mu | negbeta | linfrac | wfull(25) | t2m(25) | identity(128)
WCS = M + M + G + KC * KC + KC * KC + NP
# cstb (bf16): stc(128) | stall(128) | stone(1)
WCB = NP + NP + 1

FA_DVE_ROWS = 2                 # fa rows on DVE per piece (rest on GpSimd)

_CACHE = {}


def build_program():
    nc = bacc.Bacc("TRN2", target_bir_lowering=False, debug=False,
                   enable_asserts=False, num_devices=N_CORES)

    pk_d = nc.dram_tensor("pk", [P, WPK], F32, kind="ExternalInput")
    cst_d = nc.dram_tensor("cst", [NP, WCS], F32, kind="ExternalInput")
    cstb_d = nc.dram_tensor("cstb", [NP, WCB], BF16, kind="ExternalInput")
    rws_d = nc.dram_tensor("rws", [P, E], F32, kind="ExternalInput")
    rwt_d = nc.dram_tensor("rwt", [E, P], BF16, kind="ExternalInput")
    ut_d = nc.dram_tensor("ut", [E, S, P], F32, kind="ExternalInput")
    res_d = nc.dram_tensor("res", [P, S], F32, kind="ExternalOutput")
    DBG = os.environ.get("K_DBG") == "1"
    if DBG:
        mdbg_d = nc.dram_tensor("mdbg", [NP, RS, P], F32, kind="ExternalOutput")
        cdbg_d = nc.dram_tensor("cdbg", [NP, RS * P], F32, kind="ExternalOutput")
        idbg_d = nc.dram_tensor("idbg", [NP, RS, P], F32, kind="ExternalOutput")
        vdbg_d = nc.dram_tensor("vdbg", [1, RS * P], F32, kind="ExternalOutput")
        wdbg_d = nc.dram_tensor("wdbg", [S, P], F32, kind="ExternalOutput")

    alu = mybir.AluOpType
    act = mybir.ActivationFunctionType

    with tile.TileContext(nc) as tc:
        with (
            tc.tile_pool(name="io", bufs=1) as iop,
            tc.tile_pool(name="ubuf", bufs=1) as ubuf,
            tc.tile_pool(name="mbuf", bufs=1) as mbuf,
            tc.tile_pool(name="ph1", bufs=2) as ph1,
            tc.tile_pool(name="p2", bufs=4) as p2,
            tc.tile_pool(name="dbgp", bufs=1) as dbgp,
            tc.tile_pool(name="psA", bufs=2, space="PSUM") as psa,
            tc.tile_pool(name="psV", bufs=2, space="PSUM") as psv,
        ):
            # ---- DMAs: small packed inputs first, then the u stream ----
            pk_t = iop.tile([NP, NCH, WPK], F32, tag="pk")
            nc.sync.dma_start(out=pk_t[:],
                              in_=pk_d.ap().rearrange("(c p) w -> p c w", p=NP))
            cst_t = iop.tile([NP, WCS], F32, tag="cst")
            nc.sync.dma_start(out=cst_t[:], in_=cst_d.ap())
            cstb_t = iop.tile([NP, WCB], BF16, tag="cstb")
            nc.sync.dma_start(out=cstb_t[:], in_=cstb_d.ap())
            rws_t = iop.tile([NP, NCH, E], F32, tag="rws")
            nc.sync.dma_start(out=rws_t[:],
                              in_=rws_d.ap().rearrange("(c p) e -> p c e", p=NP))
            rwt_t = iop.tile([NP, ECH, P], BF16, tag="rwt")
            nc.sync.dma_start(out=rwt_t[:],
                              in_=rwt_d.ap().rearrange("(c a) p -> a c p", a=NP))

            ut = [ubuf.tile([NP, S, P], F32, tag=f"ut{a}", name=f"ut{a}")
                  for a in range(ECH)]
            DRS = 4
            for j in range(S // DRS):
                for a in range(ECH):
                    sl = slice(j * DRS, (j + 1) * DRS)
                    nc.sync.dma_start(
                        out=ut[a][:, sl, :],
                        in_=ut_d.ap().rearrange("(c a) s p -> a c s p", a=NP)
                            [:, a, sl, :])

            mu_t = cst_t[:, 0:M]
            negb_t = cst_t[:, M:2 * M]
            linfrac_t = cst_t[:, 2 * M:2 * M + G]
            o = 2 * M + G
            wfull_t = cst_t[:, o:o + KC * KC].rearrange("p (a b) -> p a b", a=KC)
            t2m_t = cst_t[:, o + KC * KC:o + 2 * KC * KC].rearrange(
                "p (a b) -> p a b", a=KC)
            ident_t = cst_t[:, o + 2 * KC * KC:]
            stc_t = cstb_t[:, 0:NP]
            stall_t = cstb_t[:, NP:2 * NP]
            stone_t = cstb_t[:, 2 * NP:2 * NP + 1]

            biasm = iop.tile([NP, 1], F32, tag="biasm")
            nc.gpsimd.memset(biasm[:], -BIG)

            negE = iop.tile([NP, G, M], F32, tag="negE")
            nc.vector.tensor_tensor(
                out=negE[:],
                in0=linfrac_t.unsqueeze(2).to_broadcast((NP, G, M)),
                in1=negb_t.unsqueeze(1).to_broadcast((NP, G, M)), op=alu.mult)

            # qT[a]: [e-part, pair(c major)] via PE transpose + copy to SBUF
            qT = iop.tile([NP, ECH, NP * NCH], F32, tag="qT")

            ch = [dict() for _ in range(NCH)]
            for c in range(NCH):
                d = ch[c]
                aemb = pk_t[:, c, 0:M]
                tds = pk_t[:, c, M:M + 1]
                ddom = pk_t[:, c, M + 1:M + 2]
                fourd = pk_t[:, c, M + 2:M + 3]
                d["dtb"] = pk_t[:, c, M + 3:M + 4]
                d["lastraw"] = pk_t[:, c, M + 4:M + 5]
                raw = rws_t[:, c, :]

                eng = nc.vector
                dG = ph1.tile([NP, G, M], F32, tag="gA", name=f"dG{c}")
                nc.scalar.activation(dG[:, 0:S0, :], negE[:, 0:S0, :], act.Exp,
                                     scale=tds)
                nc.scalar.activation(dG[:, S0:G, :], negE[:, S0:G, :], act.Exp,
                                     scale=ddom)
                gG = ph1.tile([NP, G, M], F32, tag="gB", name=f"gG{c}")
                eng.tensor_tensor(out=gG[:], in0=dG[:],
                                  in1=aemb.unsqueeze(1).to_broadcast((NP, G, M)),
                                  op=alu.mult)
                sG = ph1.tile([NP, G, M], F32, tag="gA", name=f"sG{c}")
                eng.tensor_tensor(out=sG[:], in0=gG[:],
                                  in1=mu_t.unsqueeze(1).to_broadcast((NP, G, M)),
                                  op=alu.add)
                eG = ph1.tile([NP, G, M], F32, tag="gB", name=f"eG{c}")
                nc.scalar.activation(eG[:], sG[:], act.Exp)
                spG = ph1.tile([NP, G, M], F32, tag="gA", name=f"spG{c}")
                nc.scalar.activation(spG[:], eG[:], act.Ln, bias=1.0)
                vals = ph1.tile([NP, G], F32, tag="vals", name=f"vals{c}")
                nc.vector.reduce_sum(out=vals[:], in_=spG[:],
                                     axis=mybir.AxisListType.X)

                bmax = ph1.tile([NP, 1], F32, tag="bmax", name=f"bmax{c}")
                nc.vector.reduce_max(out=bmax[:], in_=vals[:, 0:S0],
                                     axis=mybir.AxisListType.X)
                b15 = ph1.tile([NP, 1], F32, tag="b15", name=f"b15{c}")
                nc.vector.tensor_scalar(out=b15[:], in0=bmax[:],
                                        scalar1=float(OVER), scalar2=None,
                                        op0=alu.mult)
                invb = ph1.tile([NP, 1], F32, tag="invb", name=f"invb{c}")
                nc.vector.reciprocal(invb[:], b15[:])
                svc2 = ph1.tile([NP, 1], F32, tag="svc2", name=f"svc2{c}")
                nc.vector.tensor_scalar(out=svc2[:], in0=invb[:], scalar1=fourd,
                                        scalar2=None, op0=alu.mult)
                w2 = ph1.tile([NP, E], F32, tag="w2", name=f"w2{c}")
                nc.vector.tensor_scalar(out=w2[:], in0=raw, scalar1=svc2[:],
                                        scalar2=-2.0, op0=alu.mult, op1=alu.add)

                cw = ph1.tile([NP, KC, KC], F32, tag="cw", name=f"cw{c}")
                nc.vector.tensor_tensor(
                    out=cw[:],
                    in0=vals[:, S0:G].unsqueeze(1).to_broadcast((NP, KC, KC)),
                    in1=wfull_t, op=alu.mult)
                cc = ph1.tile([NP, KC], F32, tag="cc", name=f"cc{c}")
                nc.vector.reduce_sum(out=cc[:], in_=cw[:],
                                     axis=mybir.AxisListType.X)
                cw2 = ph1.tile([NP, KC, KC], F32, tag="cw2", name=f"cw2{c}")
                nc.vector.tensor_tensor(
                    out=cw2[:],
                    in0=cc[:].unsqueeze(1).to_broadcast((NP, KC, KC)),
                    in1=t2m_t, op=alu.mult)
                am = ph1.tile([NP, KC], F32, tag="am", name=f"am{c}")
                nc.vector.reduce_sum(out=am[:], in_=cw2[:],
                                     axis=mybir.AxisListType.X)

                x2 = ph1.tile([NP, E], F32, tag="x2", name=f"x2{c}")
                nc.vector.tensor_tensor(out=x2[:], in0=w2[:], in1=w2[:],
                                        op=alu.mult)
                u1 = ph1.tile([NP, E], F32, tag="u1", name=f"u1{c}")
                nc.vector.tensor_scalar(out=u1[:], in0=x2[:],
                                        scalar1=am[:, 4:5], scalar2=am[:, 2:3],
                                        op0=alu.mult, op1=alu.add)
                u2 = ph1.tile([NP, E], F32, tag="u2", name=f"u2{c}")
                nc.vector.tensor_tensor(out=u2[:], in0=u1[:], in1=x2[:],
                                        op=alu.mult)
                v1 = ph1.tile([NP, E], F32, tag="v1", name=f"v1{c}")
                nc.vector.tensor_scalar(out=v1[:], in0=x2[:],
                                        scalar1=am[:, 3:4], scalar2=am[:, 1:2],
                                        op0=alu.mult, op1=alu.add)
                v2 = ph1.tile([NP, E], F32, tag="v2", name=f"v2{c}")
                nc.vector.tensor_tensor(out=v2[:], in0=v1[:], in1=w2[:],
                                        op=alu.mult)
                tot = ph1.tile([NP, E], F32, tag="tot", name=f"tot{c}")
                nc.vector.scalar_tensor_tensor(out=tot[:], in0=u2[:],
                                               scalar=am[:, 0:1], in1=v2[:],
                                               op0=alu.add, op1=alu.add)
                q = ph1.tile([NP, E], F32, tag="q", name=f"q{c}")
                nc.vector.tensor_scalar(out=q[:], in0=tot[:], scalar1=invb[:],
                                        scalar2=None, op0=alu.mult)
                # q [pair, e] -> qT [e, pair] (PE transpose + SBUF copy)
                for a in range(ECH):
                    qsc = psa.tile([NP, NPC], F32, tag="cnt", name=f"qsc{c}_{a}")
                    nc.tensor.transpose(qsc[:, 0:NP],
                                        q[:, a * NP:(a + 1) * NP], ident_t)
                    nc.scalar.activation(qT[:, a, c * NP:(c + 1) * NP],
                                          qsc[:, 0:NP], act.Copy)
                d.update(invb=invb)

            # ---- phase 2: 16 pieces ----
            vst = iop.tile([S, P], F32, tag="vst")
            m_t = [mbuf.tile([NP, S, P], BF16, tag=f"m{a}", name=f"m{a}")
                   for a in range(ECH)]
            for j in range(NSP):
                sl = slice(j * RS, (j + 1) * RS)
                for a in range(ECH):
                    nc.vector.tensor_tensor(
                        out=m_t[a][:, sl, :], in0=ut[a][:, sl, :],
                        in1=qT[:, a, :].unsqueeze(1).to_broadcast((NP, RS, P)),
                        op=alu.is_lt)
                    mv = m_t[a][:, sl, :].rearrange("a s p -> a (s p)")
                    cnt = psa.tile([NP, NPC], F32, tag="cnt", name=f"cnt{a}_{j}")
                    HH = NPC // 2
                    for h in range(2):
                        hs = slice(h * HH, (h + 1) * HH)
                        if a == 0:
                            nc.tensor.matmul(cnt[:, hs], stc_t, mv[:, hs],
                                             start=True, stop=True)
                        else:
                            nc.tensor.matmul(cnt[:, hs], stc_t, mv[:, hs],
                                             start=True, stop=False)
                            nc.tensor.matmul(
                                cnt[:, hs], stall_t,
                                m_t[0][:, sl, :].rearrange("a s p -> a (s p)")[:, hs],
                                start=False, stop=True)
                    ind = p2.tile([NP, RS, P], BF16, tag="ind", name=f"ind{a}_{j}")
                    nc.scalar.activation(ind[:].rearrange("a s p -> a (s p)"),
                                         cnt[:], act.Exp, bias=biasm[:])
                    rbc = rwt_t[:, a, :].unsqueeze(1)
                    fd = FA_DVE_ROWS
                    fa = p2.tile([NP, RS, P], BF16, tag="fa", name=f"fa{a}_{j}")
                    nc.vector.tensor_tensor(
                        out=fa[:, 0:fd, :], in0=ind[:, 0:fd, :],
                        in1=rbc.to_broadcast((NP, fd, P)), op=alu.mult)
                    nc.gpsimd.tensor_tensor(
                        out=fa[:, fd:RS, :], in0=ind[:, fd:RS, :],
                        in1=rbc.to_broadcast((NP, RS - fd, P)), op=alu.mult)
                    if a == 0:
                        pv = psv.tile([1, NPC], F32, tag="pv", name=f"pv{j}")
                        pv_hold = pv
                    else:
                        pv = pv_hold
                    fav = fa[:].rearrange("a s p -> a (s p)")
                    for h in range(2):
                        hs = slice(h * (NPC // 2), (h + 1) * (NPC // 2))
                        nc.tensor.matmul(pv[:, hs], stone_t, fav[:, hs],
                                         start=(a == 0), stop=(a == 1))
                    if DBG and j == 1 and a == 0:
                        t1 = dbgp.tile([NP, RS, P], F32, tag="dbg1")
                        nc.vector.tensor_copy(t1[:], m_t[a][:, sl, :])
                        nc.sync.dma_start(out=mdbg_d.ap(), in_=t1[:])
                        t2 = dbgp.tile([NP, RS * P], F32, tag="dbg2")
                        nc.vector.tensor_copy(t2[:], cnt[:])
                        nc.sync.dma_start(out=cdbg_d.ap(), in_=t2[:])
                        t3 = dbgp.tile([NP, RS, P], F32, tag="dbg3")
                        nc.vector.tensor_copy(t3[:], ind[:])
                        nc.sync.dma_start(out=idbg_d.ap(), in_=t3[:])
                    if DBG and j == 1 and a == 1:
                        t4 = dbgp.tile([1, RS * P], F32, tag="dbg4")
                        nc.vector.tensor_copy(t4[:], pv[:])
                        nc.sync.dma_start(out=vdbg_d.ap(), in_=t4[:])
                    if a == 1:
                        svj = p2.tile([1, NPC], F32, tag=f"sv{j % 2}",
                                      name=f"sv{j}")
                        nc.scalar.activation(svj[:], pv[:], act.Copy)
                        nc.sync.dma_start(
                            out=vst[j * RS:(j + 1) * RS, :],
                            in_=svj[:].rearrange("o (s p) -> o s p", s=RS))

            if DBG:
                nc.sync.dma_start(out=wdbg_d.ap(), in_=vst[:])
            # PE-transpose the collected val rows back to pair-layout
            valT = psa.tile([NP, NPC], F32, tag="cnt", name="valT")
            for c in range(NCH):
                nc.tensor.transpose(valT[:, c * S:(c + 1) * S],
                                    vst[:, c * NP:(c + 1) * NP],
                                    ident_t[0:S, 0:S])

            resall = ph1.tile([NP, NCH, S], F32, tag="resall")
            whoall = ph1.tile([NP, NCH * S], mybir.dt.int32, tag="whoall")
            nc.vector.tensor_scalar(out=whoall[:], in0=valT[:, 0:NCH * S],
                                    scalar1=0.0, scalar2=None, op0=alu.is_gt)
            for c in range(NCH):
                d = ch[c]
                invb = d["invb"]
                val = valT[:, c * S:(c + 1) * S]
                acc = ph1.tile([NP, S], F32, tag="acc", name=f"acc{c}")
                nc.vector.tensor_scalar(out=acc[:], in0=val, scalar1=invb[:],
                                        scalar2=None, op0=alu.mult)
                lastx = ph1.tile([NP, 1], F32, tag="lastx", name=f"lastx{c}")
                nc.vector.tensor_scalar(out=lastx[:], in0=d["lastraw"],
                                        scalar1=invb[:], scalar2=None,
                                        op0=alu.mult)
                fb = ph1.tile([NP, 1], F32, tag="fb", name=f"fb{c}")
                nc.vector.tensor_tensor(out=fb[:], in0=lastx[:], in1=d["dtb"],
                                        op=alu.max)
                nc.scalar.activation(resall[:, c, :],
                                     fb[:].to_broadcast((NP, S)), act.Copy)
                nc.vector.copy_predicated(resall[:, c, :],
                                          whoall[:, c * S:(c + 1) * S], acc[:])
            res2_t = ph1.tile([NP, NCH, S], F32, tag="res2")
            nc.vector.tensor_scalar(out=res2_t[:], in0=resall[:], scalar1=1e5,
                                    scalar2=None, op0=alu.min)
            nc.sync.dma_start(
                out=res_d.ap().rearrange("(c p) s -> p c s", p=NP),
                in_=res2_t[:])

    nc.finalize()
    return nc


def _prep_inputs(time_seq, time_delta_seq, event_seq, dtime_boundary, exp_raw,
                 unif_numbers, mu, alpha, beta, type_emb):
    f = np.float32
    tds = np.asarray(time_delta_seq, f).reshape(B * L)
    dtb = np.asarray(dtime_boundary, f).reshape(B * L)
    raw0 = np.asarray(exp_raw, f).reshape(B * L, E)
    u = np.asarray(unif_numbers, f).reshape(B * L, S, E)
    ev = np.asarray(event_seq)
    mu = np.asarray(mu, f)
    alpha = np.asarray(alpha, f)
    beta = np.asarray(beta, f)
    type_emb = np.asarray(type_emb, f)

    aemb = (alpha[None, :] * type_emb)[ev].reshape(B * L, M).astype(f)

    order = np.argsort(raw0, axis=-1, kind="stable")
    raws = np.take_along_axis(raw0, order, axis=-1).astype(f)
    us = np.take_along_axis(u, order[:, None, :], axis=-1).astype(f)

    tot00 = np.log1p(np.exp((aemb + mu[None, :]).astype(np.float64))).sum(-1)
    rawmax = raw0.max(-1).astype(np.float64)
    Ddom = rawmax / (1.5 * tot00)
    fourd = (4.0 / Ddom).astype(f)
    ddom = Ddom.astype(f)

    jj = np.arange(KC)
    n = KC - 1
    frac = (1.0 + np.cos(np.pi * jj / n)) / 2.0
    linfrac = np.concatenate([np.linspace(0.0, 1.0, S0), frac]).astype(f)

    Wm = np.zeros((KC, KC))
    for k in range(KC):
        wrow = np.cos(np.pi * jj * k / n)
        wrow[0] *= 0.5
        wrow[-1] *= 0.5
        wrow *= 2.0 / n
        if k == 0 or k == n:
            wrow *= 0.5
        Wm[k] = wrow
    # tot = sum_k cc_k T_k(w2/2); T_k(w2/2) as powers of w2 (cols = power)
    t2m = np.zeros((KC, KC))
    t2m[0, 0] = 1.0
    t2m[1, 1] = 0.5
    t2m[2, 0], t2m[2, 2] = -1.0, 0.5
    t2m[3, 1], t2m[3, 3] = -1.5, 0.5
    t2m[4, 0], t2m[4, 2], t2m[4, 4] = 1.0, -2.0, 0.5
    # cw2 uses cc broadcast over rows a: am_j = sum_b cc_b * t2m[b, j]
    t2m_packed = t2m.T.reshape(1, KC * KC)  # [a=j(power), b=k(cheb)] row-major

    def bf16_bytes(x):
        x = np.ascontiguousarray(np.asarray(x, np.float32))
        u32 = x.view(np.uint32)
        r = ((u32 + 0x7FFF + ((u32 >> 16) & 1)) >> 16).astype(np.uint16)
        return r

    cst = np.concatenate([
        np.tile(mu[None, :], (NP, 1)),
        np.tile(-beta[None, :], (NP, 1)),
        np.tile(linfrac[None, :], (NP, 1)),
        np.tile(Wm.reshape(1, KC * KC).astype(f), (NP, 1)),
        np.tile(t2m_packed.astype(f), (NP, 1)),
        np.eye(NP, dtype=f),
    ], axis=1).astype(f)

    stc = np.zeros((NP, NP), f)
    for k in range(NP):
        stc[k, k] = BIG
        stc[:k, k] = -BIG
    stall = np.full((NP, NP), -BIG, f)
    stone = np.ones((NP, 1), f)
    cstb = np.concatenate([bf16_bytes(stc), bf16_bytes(stall),
                           bf16_bytes(stone)], axis=1)

    pk = np.concatenate([
        aemb, tds[:, None], ddom[:, None], fourd[:, None], dtb[:, None],
        raw0[:, E - 1:E],
    ], axis=1).astype(f)

    in_maps = []
    for core in range(N_CORES):
        rs = slice(core * P, (core + 1) * P)
        uT = np.ascontiguousarray(us[rs].transpose(2, 1, 0))   # [E, S, P]
        rwt = bf16_bytes(raws[rs].T)                           # [E, P] bf16
        in_maps.append(dict(
            pk=np.ascontiguousarray(pk[rs]),
            cst=cst,
            cstb=cstb,
            rws=np.ascontiguousarray(raws[rs]),
            rwt=np.ascontiguousarray(rwt),
            ut=uT,
        ))
    return in_maps


def kernel(time_seq, time_delta_seq, event_seq, dtime_boundary, exp_raw,
           unif_numbers, mu, alpha, beta, type_emb, _trace=False):
    if "nc" not in _CACHE:
        _CACHE["nc"] = build_program()
    nc = _CACHE["nc"]

    in_maps = _prep_inputs(time_seq, time_delta_seq, event_seq, dtime_boundary,
                           exp_raw, unif_numbers, mu, alpha, beta, type_emb)

    out = run_bass_kernel_spmd(nc, in_maps, core_ids=list(range(N_CORES)),
                               trace=_trace)
    _CACHE["last_results"] = out

    res = np.concatenate([out.results[c]["res"].reshape(BPC, L, S)
                          for c in range(N_CORES)], axis=0)
    weights = np.full((B, L, S), np.float32(1.0 / S), np.float32)
    return res, weights


# revision 42
# speedup vs baseline: 1.0597x; 1.0597x over previous
"""Trainium2 Bass kernel for nn_EventSampler (thinning / rejection sampling).

kernel(**inputs) takes FULL unsharded inputs, shards batch across 8 cores
(2 batches = 256 (b,l) pairs per core), runs one SPMD Bass program, returns
the full output.

v3 structure (cost-model driven; per core):
  host prep: e-axis of (exp_raw, unif_numbers) sorted ascending by exp_raw
  per (b,l) pair, u transposed to [E, S, P] so the accept test runs in an
  e-on-partition layout.

  phase 1 (pair-layout, f32, same operation classes as the validated
  baseline): 25-point grid (20 bound-scan + 5 Chebyshev-Lobatto nodes) ->
  bounds -> b15/invb -> degree-4 polynomial tot(x) at the sorted draws ->
  q = tot*invb.  q is transposed to e-layout via PE transpose.

  phase 2 (e-layout, 16 pipelined pieces = 2 e-chunks x 8 4-row s-slices):
    m = (u < q) on DVE (the only full-size f32 pass),
    first-accept extraction on PE: counts = (200*I - 200*strict_tri) @ m
    (+ cross-chunk -200*ones @ m0; 512-col matmul halves, bank-aligned so
    start=True resets don't clobber the sibling half),
    IND = Act(Exp, counts - 200) in {0,1} exactly,
    fa = IND * raw_sorted (bf16; rows split FA_DVE_ROWS DVE, rest GpSimd),
    val = ones @ fa = raw* (or exact 0 if no accept), accumulated over both
    e-chunks in PSUM, parked to SBUF via Act copy.
  val rows return to pair-layout via PE transpose; tail (who = val>0,
  acc = val*invb, fallback max(last_raw*invb, dtb), min 1e5) is tiny.

Decision-critical arithmetic stays f32; bf16 only on the value path
(res tolerance 2e-2; host-sim on the real inputs shows max rel 3.9e-3).
Cost-model timeline: 62384 ns vs 80379 ns for the previous kernel.
"""

import os
import sys

import numpy as np

for _p in ("/opt/trn_rl_repo",):
    if _p not in sys.path and os.path.isdir(_p):
        sys.path.insert(0, _p)

import concourse.bacc as bacc
import concourse.tile as tile
import concourse.mybir as mybir
from concourse.bass_utils import run_bass_kernel_spmd

F32 = mybir.dt.float32
BF16 = mybir.dt.bfloat16

B, L, M = 16, 128, 32
S, E, S0 = 32, 256, 20
OVER = 1.5
KC = 5
G = S0 + KC
N_CORES = 8
BPC = B // N_CORES
P = BPC * L                     # pairs per core = 256
NP = 128
NCH = P // NP                   # pair chunks = 2
ECH = E // NP                   # e chunks = 2
RS = 4                          # s-rows per phase-2 piece
NSP = S // RS                   # s-pieces = 8
NPC = RS * P                    # free cols per piece = 1024
BIG = 200.0

# pk layout: aemb | tds | ddom | fourd | dtb | lastraw
WPK = M + 5
# cst (f32): mu | negbeta | linfrac | wfull(25) | t2m(25) | identity(128)
WCS = M + M + G + KC * KC + KC * KC + NP
# cstb (bf16): stc(128) | stall(128) | stone(1)
WCB = NP + NP + 1

FA_DVE_ROWS = 3                 # fa rows on DVE per piece (rest on GpSimd)

_CACHE = {}


def build_program():
    nc = bacc.Bacc("TRN2", target_bir_lowering=False, debug=False,
                   enable_asserts=False, num_devices=N_CORES)

    pk_d = nc.dram_tensor("pk", [P, WPK], F32, kind="ExternalInput")
    cst_d = nc.dram_tensor("cst", [NP, WCS], F32, kind="ExternalInput")
    cstb_d = nc.dram_tensor("cstb", [NP, WCB], BF16, kind="ExternalInput")
    rws_d = nc.dram_tensor("rws", [P, E], F32, kind="ExternalInput")
    rwt_d = nc.dram_tensor("rwt", [E, P], BF16, kind="ExternalInput")
    ut_d = nc.dram_tensor("ut", [E, S, P], F32, kind="ExternalInput")
    res_d = nc.dram_tensor("res", [P, S], F32, kind="ExternalOutput")
    DBG = os.environ.get("K_DBG") == "1"
    if DBG:
        mdbg_d = nc.dram_tensor("mdbg", [NP, RS, P], F32, kind="ExternalOutput")
        cdbg_d = nc.dram_tensor("cdbg", [NP, RS * P], F32, kind="ExternalOutput")
        idbg_d = nc.dram_tensor("idbg", [NP, RS, P], F32, kind="ExternalOutput")
        vdbg_d = nc.dram_tensor("vdbg", [1, RS * P], F32, kind="ExternalOutput")
        wdbg_d = nc.dram_tensor("wdbg", [S, P], F32, kind="ExternalOutput")

    alu = mybir.AluOpType
    act = mybir.ActivationFunctionType

    with tile.TileContext(nc) as tc:
        with (
            tc.tile_pool(name="io", bufs=1) as iop,
            tc.tile_pool(name="ubuf", bufs=1) as ubuf,
            tc.tile_pool(name="mbuf", bufs=1) as mbuf,
            tc.tile_pool(name="ph1", bufs=2) as ph1,
            tc.tile_pool(name="p2", bufs=4) as p2,
            tc.tile_pool(name="dbgp", bufs=1) as dbgp,
            tc.tile_pool(name="psA", bufs=2, space="PSUM") as psa,
            tc.tile_pool(name="psV", bufs=2, space="PSUM") as psv,
        ):
            # ---- DMAs: small packed inputs first, then the u stream ----
            pk_t = iop.tile([NP, NCH, WPK], F32, tag="pk")
            nc.sync.dma_start(out=pk_t[:],
                              in_=pk_d.ap().rearrange("(c p) w -> p c w", p=NP))
            cst_t = iop.tile([NP, WCS], F32, tag="cst")
            nc.sync.dma_start(out=cst_t[:], in_=cst_d.ap())
            cstb_t = iop.tile([NP, WCB], BF16, tag="cstb")
            nc.sync.dma_start(out=cstb_t[:], in_=cstb_d.ap())
            rws_t = iop.tile([NP, NCH, E], F32, tag="rws")
            nc.sync.dma_start(out=rws_t[:],
                              in_=rws_d.ap().rearrange("(c p) e -> p c e", p=NP))
            rwt_t = iop.tile([NP, ECH, P], BF16, tag="rwt")
            nc.sync.dma_start(out=rwt_t[:],
                              in_=rwt_d.ap().rearrange("(c a) p -> a c p", a=NP))

            ut = [ubuf.tile([NP, S, P], F32, tag=f"ut{a}", name=f"ut{a}")
                  for a in range(ECH)]
            DRS = 4
            for j in range(S // DRS):
                for a in range(ECH):
                    sl = slice(j * DRS, (j + 1) * DRS)
                    nc.sync.dma_start(
                        out=ut[a][:, sl, :],
                        in_=ut_d.ap().rearrange("(c a) s p -> a c s p", a=NP)
                            [:, a, sl, :])

            mu_t = cst_t[:, 0:M]
            negb_t = cst_t[:, M:2 * M]
            linfrac_t = cst_t[:, 2 * M:2 * M + G]
            o = 2 * M + G
            wfull_t = cst_t[:, o:o + KC * KC].rearrange("p (a b) -> p a b", a=KC)
            t2m_t = cst_t[:, o + KC * KC:o + 2 * KC * KC].rearrange(
                "p (a b) -> p a b", a=KC)
            ident_t = cst_t[:, o + 2 * KC * KC:]
            stc_t = cstb_t[:, 0:NP]
            stall_t = cstb_t[:, NP:2 * NP]
            stone_t = cstb_t[:, 2 * NP:2 * NP + 1]

            biasm = iop.tile([NP, 1], F32, tag="biasm")
            nc.gpsimd.memset(biasm[:], -BIG)

            negE = iop.tile([NP, G, M], F32, tag="negE")
            nc.vector.tensor_tensor(
                out=negE[:],
                in0=linfrac_t.unsqueeze(2).to_broadcast((NP, G, M)),
                in1=negb_t.unsqueeze(1).to_broadcast((NP, G, M)), op=alu.mult)

            # qT[a]: [e-part, pair(c major)] via PE transpose + copy to SBUF
            qT = iop.tile([NP, ECH, NP * NCH], F32, tag="qT")

            ch = [dict() for _ in range(NCH)]
            for c in range(NCH):
                d = ch[c]
                aemb = pk_t[:, c, 0:M]
                tds = pk_t[:, c, M:M + 1]
                ddom = pk_t[:, c, M + 1:M + 2]
                fourd = pk_t[:, c, M + 2:M + 3]
                d["dtb"] = pk_t[:, c, M + 3:M + 4]
                d["lastraw"] = pk_t[:, c, M + 4:M + 5]
                raw = rws_t[:, c, :]

                eng = nc.vector
                dG = ph1.tile([NP, G, M], F32, tag="gA", name=f"dG{c}")
                nc.scalar.activation(dG[:, 0:S0, :], negE[:, 0:S0, :], act.Exp,
                                     scale=tds)
                nc.scalar.activation(dG[:, S0:G, :], negE[:, S0:G, :], act.Exp,
                                     scale=ddom)
                gG = ph1.tile([NP, G, M], F32, tag="gB", name=f"gG{c}")
                eng.tensor_tensor(out=gG[:], in0=dG[:],
                                  in1=aemb.unsqueeze(1).to_broadcast((NP, G, M)),
                                  op=alu.mult)
                sG = ph1.tile([NP, G, M], F32, tag="gA", name=f"sG{c}")
                eng.tensor_tensor(out=sG[:], in0=gG[:],
                                  in1=mu_t.unsqueeze(1).to_broadcast((NP, G, M)),
                                  op=alu.add)
                eG = ph1.tile([NP, G, M], F32, tag="gB", name=f"eG{c}")
                nc.scalar.activation(eG[:], sG[:], act.Exp)
                spG = ph1.tile([NP, G, M], F32, tag="gA", name=f"spG{c}")
                nc.scalar.activation(spG[:], eG[:], act.Ln, bias=1.0)
                vals = ph1.tile([NP, G], F32, tag="vals", name=f"vals{c}")
                nc.vector.reduce_sum(out=vals[:], in_=spG[:],
                                     axis=mybir.AxisListType.X)

                bmax = ph1.tile([NP, 1], F32, tag="bmax", name=f"bmax{c}")
                nc.vector.reduce_max(out=bmax[:], in_=vals[:, 0:S0],
                                     axis=mybir.AxisListType.X)
                b15 = ph1.tile([NP, 1], F32, tag="b15", name=f"b15{c}")
                nc.vector.tensor_scalar(out=b15[:], in0=bmax[:],
                                        scalar1=float(OVER), scalar2=None,
                                        op0=alu.mult)
                invb = ph1.tile([NP, 1], F32, tag="invb", name=f"invb{c}")
                nc.vector.reciprocal(invb[:], b15[:])
                svc2 = ph1.tile([NP, 1], F32, tag="svc2", name=f"svc2{c}")
                nc.vector.tensor_scalar(out=svc2[:], in0=invb[:], scalar1=fourd,
                                        scalar2=None, op0=alu.mult)
                w2 = ph1.tile([NP, E], F32, tag="w2", name=f"w2{c}")
                nc.vector.tensor_scalar(out=w2[:], in0=raw, scalar1=svc2[:],
                                        scalar2=-2.0, op0=alu.mult, op1=alu.add)

                cw = ph1.tile([NP, KC, KC], F32, tag="cw", name=f"cw{c}")
                nc.vector.tensor_tensor(
                    out=cw[:],
                    in0=vals[:, S0:G].unsqueeze(1).to_broadcast((NP, KC, KC)),
                    in1=wfull_t, op=alu.mult)
                cc = ph1.tile([NP, KC], F32, tag="cc", name=f"cc{c}")
                nc.vector.reduce_sum(out=cc[:], in_=cw[:],
                                     axis=mybir.AxisListType.X)
                cw2 = ph1.tile([NP, KC, KC], F32, tag="cw2", name=f"cw2{c}")
                nc.vector.tensor_tensor(
                    out=cw2[:],
                    in0=cc[:].unsqueeze(1).to_broadcast((NP, KC, KC)),
                    in1=t2m_t, op=alu.mult)
                am = ph1.tile([NP, KC], F32, tag="am", name=f"am{c}")
                nc.vector.reduce_sum(out=am[:], in_=cw2[:],
                                     axis=mybir.AxisListType.X)

                x2 = ph1.tile([NP, E], F32, tag="x2", name=f"x2{c}")
                nc.vector.tensor_tensor(out=x2[:], in0=w2[:], in1=w2[:],
                                        op=alu.mult)
                u1 = ph1.tile([NP, E], F32, tag="u1", name=f"u1{c}")
                nc.vector.tensor_scalar(out=u1[:], in0=x2[:],
                                        scalar1=am[:, 4:5], scalar2=am[:, 2:3],
                                        op0=alu.mult, op1=alu.add)
                u2 = ph1.tile([NP, E], F32, tag="u2", name=f"u2{c}")
                nc.vector.tensor_tensor(out=u2[:], in0=u1[:], in1=x2[:],
                                        op=alu.mult)
                v1 = ph1.tile([NP, E], F32, tag="v1", name=f"v1{c}")
                nc.vector.tensor_scalar(out=v1[:], in0=x2[:],
                                        scalar1=am[:, 3:4], scalar2=am[:, 1:2],
                                        op0=alu.mult, op1=alu.add)
                v2 = ph1.tile([NP, E], F32, tag="v2", name=f"v2{c}")
                nc.vector.tensor_tensor(out=v2[:], in0=v1[:], in1=w2[:],
                                        op=alu.mult)
                tot = ph1.tile([NP, E], F32, tag="tot", name=f"tot{c}")
                nc.vector.scalar_tensor_tensor(out=tot[:], in0=u2[:],
                                               scalar=am[:, 0:1], in1=v2[:],
                                               op0=alu.add, op1=alu.add)
                q = ph1.tile([NP, E], F32, tag="q", name=f"q{c}")
                nc.vector.tensor_scalar(out=q[:], in0=tot[:], scalar1=invb[:],
                                        scalar2=None, op0=alu.mult)
                # q [pair, e] -> qT [e, pair] (PE transpose + SBUF copy)
                for a in range(ECH):
                    qsc = psa.tile([NP, NPC], F32, tag="cnt", name=f"qsc{c}_{a}")
                    nc.tensor.transpose(qsc[:, 0:NP],
                                        q[:, a * NP:(a + 1) * NP], ident_t)
                    nc.scalar.activation(qT[:, a, c * NP:(c + 1) * NP],
                                          qsc[:, 0:NP], act.Copy)
                d.update(invb=invb)

            # ---- phase 2: 16 pieces ----
            vst = iop.tile([S, P], F32, tag="vst")
            m_t = [mbuf.tile([NP, S, P], BF16, tag=f"m{a}", name=f"m{a}")
                   for a in range(ECH)]
            for j in range(NSP):
                sl = slice(j * RS, (j + 1) * RS)
                for a in range(ECH):
                    nc.vector.tensor_tensor(
                        out=m_t[a][:, sl, :], in0=ut[a][:, sl, :],
                        in1=qT[:, a, :].unsqueeze(1).to_broadcast((NP, RS, P)),
                        op=alu.is_lt)
                    mv = m_t[a][:, sl, :].rearrange("a s p -> a (s p)")
                    cnt = psa.tile([NP, NPC], F32, tag="cnt", name=f"cnt{a}_{j}")
                    HH = NPC // 2
                    for h in range(2):
                        hs = slice(h * HH, (h + 1) * HH)
                        if a == 0:
                            nc.tensor.matmul(cnt[:, hs], stc_t, mv[:, hs],
                                             start=True, stop=True)
                        else:
                            nc.tensor.matmul(cnt[:, hs], stc_t, mv[:, hs],
                                             start=True, stop=False)
                            nc.tensor.matmul(
                                cnt[:, hs], stall_t,
                                m_t[0][:, sl, :].rearrange("a s p -> a (s p)")[:, hs],
                                start=False, stop=True)
                    ind = p2.tile([NP, RS, P], BF16, tag="ind", name=f"ind{a}_{j}")
                    nc.scalar.activation(ind[:].rearrange("a s p -> a (s p)"),
                                         cnt[:], act.Exp, bias=biasm[:])
                    rbc = rwt_t[:, a, :].unsqueeze(1)
                    fd = FA_DVE_ROWS
                    fa = p2.tile([NP, RS, P], BF16, tag="fa", name=f"fa{a}_{j}")
                    nc.vector.tensor_tensor(
                        out=fa[:, 0:fd, :], in0=ind[:, 0:fd, :],
                        in1=rbc.to_broadcast((NP, fd, P)), op=alu.mult)
                    nc.gpsimd.tensor_tensor(
                        out=fa[:, fd:RS, :], in0=ind[:, fd:RS, :],
                        in1=rbc.to_broadcast((NP, RS - fd, P)), op=alu.mult)
                    if a == 0:
                        pv = psv.tile([1, NPC], F32, tag="pv", name=f"pv{j}")
                        pv_hold = pv
                    else:
                        pv = pv_hold
                    fav = fa[:].rearrange("a s p -> a (s p)")
                    for h in range(2):
                        hs = slice(h * (NPC // 2), (h + 1) * (NPC // 2))
                        nc.tensor.matmul(pv[:, hs], stone_t, fav[:, hs],
                                         start=(a == 0), stop=(a == 1))
                    if DBG and j == 1 and a == 0:
                        t1 = dbgp.tile([NP, RS, P], F32, tag="dbg1")
                        nc.vector.tensor_copy(t1[:], m_t[a][:, sl, :])
                        nc.sync.dma_start(out=mdbg_d.ap(), in_=t1[:])
                        t2 = dbgp.tile([NP, RS * P], F32, tag="dbg2")
                        nc.vector.tensor_copy(t2[:], cnt[:])
                        nc.sync.dma_start(out=cdbg_d.ap(), in_=t2[:])
                        t3 = dbgp.tile([NP, RS, P], F32, tag="dbg3")
                        nc.vector.tensor_copy(t3[:], ind[:])
                        nc.sync.dma_start(out=idbg_d.ap(), in_=t3[:])
                    if DBG and j == 1 and a == 1:
                        t4 = dbgp.tile([1, RS * P], F32, tag="dbg4")
                        nc.vector.tensor_copy(t4[:], pv[:])
                        nc.sync.dma_start(out=vdbg_d.ap(), in_=t4[:])
                    if a == 1:
                        svj = p2.tile([1, NPC], F32, tag=f"sv{j % 2}",
                                      name=f"sv{j}")
                        nc.scalar.activation(svj[:], pv[:], act.Copy)
                        nc.sync.dma_start(
                            out=vst[j * RS:(j + 1) * RS, :],
                            in_=svj[:].rearrange("o (s p) -> o s p", s=RS))

            if DBG:
                nc.sync.dma_start(out=wdbg_d.ap(), in_=vst[:])
            # PE-transpose the collected val rows back to pair-layout
            valT = psa.tile([NP, NPC], F32, tag="cnt", name="valT")
            for c in range(NCH):
                nc.tensor.transpose(valT[:, c * S:(c + 1) * S],
                                    vst[:, c * NP:(c + 1) * NP],
                                    ident_t[0:S, 0:S])

            resall = ph1.tile([NP, NCH, S], F32, tag="resall")
            whoall = ph1.tile([NP, NCH * S], mybir.dt.int32, tag="whoall")
            nc.vector.tensor_scalar(out=whoall[:], in0=valT[:, 0:NCH * S],
                                    scalar1=0.0, scalar2=None, op0=alu.is_gt)
            for c in range(NCH):
                d = ch[c]
                invb = d["invb"]
                val = valT[:, c * S:(c + 1) * S]
                acc = ph1.tile([NP, S], F32, tag="acc", name=f"acc{c}")
                nc.vector.tensor_scalar(out=acc[:], in0=val, scalar1=invb[:],
                                        scalar2=None, op0=alu.mult)
                lastx = ph1.tile([NP, 1], F32, tag="lastx", name=f"lastx{c}")
                nc.vector.tensor_scalar(out=lastx[:], in0=d["lastraw"],
                                        scalar1=invb[:], scalar2=None,
                                        op0=alu.mult)
                fb = ph1.tile([NP, 1], F32, tag="fb", name=f"fb{c}")
                nc.vector.tensor_tensor(out=fb[:], in0=lastx[:], in1=d["dtb"],
                                        op=alu.max)
                nc.scalar.activation(resall[:, c, :],
                                     fb[:].to_broadcast((NP, S)), act.Copy)
                nc.vector.copy_predicated(resall[:, c, :],
                                          whoall[:, c * S:(c + 1) * S], acc[:])
            res2_t = ph1.tile([NP, NCH, S], F32, tag="res2")
            nc.vector.tensor_scalar(out=res2_t[:], in0=resall[:], scalar1=1e5,
                                    scalar2=None, op0=alu.min)
            nc.sync.dma_start(
                out=res_d.ap().rearrange("(c p) s -> p c s", p=NP),
                in_=res2_t[:])

    nc.finalize()
    return nc


def _prep_inputs(time_seq, time_delta_seq, event_seq, dtime_boundary, exp_raw,
                 unif_numbers, mu, alpha, beta, type_emb):
    f = np.float32
    tds = np.asarray(time_delta_seq, f).reshape(B * L)
    dtb = np.asarray(dtime_boundary, f).reshape(B * L)
    raw0 = np.asarray(exp_raw, f).reshape(B * L, E)
    u = np.asarray(unif_numbers, f).reshape(B * L, S, E)
    ev = np.asarray(event_seq)
    mu = np.asarray(mu, f)
    alpha = np.asarray(alpha, f)
    beta = np.asarray(beta, f)
    type_emb = np.asarray(type_emb, f)

    aemb = (alpha[None, :] * type_emb)[ev].reshape(B * L, M).astype(f)

    order = np.argsort(raw0, axis=-1, kind="stable")
    raws = np.take_along_axis(raw0, order, axis=-1).astype(f)
    us = np.take_along_axis(u, order[:, None, :], axis=-1).astype(f)

    tot00 = np.log1p(np.exp((aemb + mu[None, :]).astype(np.float64))).sum(-1)
    rawmax = raw0.max(-1).astype(np.float64)
    Ddom = rawmax / (1.5 * tot00)
    fourd = (4.0 / Ddom).astype(f)
    ddom = Ddom.astype(f)

    jj = np.arange(KC)
    n = KC - 1
    frac = (1.0 + np.cos(np.pi * jj / n)) / 2.0
    linfrac = np.concatenate([np.linspace(0.0, 1.0, S0), frac]).astype(f)

    Wm = np.zeros((KC, KC))
    for k in range(KC):
        wrow = np.cos(np.pi * jj * k / n)
        wrow[0] *= 0.5
        wrow[-1] *= 0.5
        wrow *= 2.0 / n
        if k == 0 or k == n:
            wrow *= 0.5
        Wm[k] = wrow
    # tot = sum_k cc_k T_k(w2/2); T_k(w2/2) as powers of w2 (cols = power)
    t2m = np.zeros((KC, KC))
    t2m[0, 0] = 1.0
    t2m[1, 1] = 0.5
    t2m[2, 0], t2m[2, 2] = -1.0, 0.5
    t2m[3, 1], t2m[3, 3] = -1.5, 0.5
    t2m[4, 0], t2m[4, 2], t2m[4, 4] = 1.0, -2.0, 0.5
    # cw2 uses cc broadcast over rows a: am_j = sum_b cc_b * t2m[b, j]
    t2m_packed = t2m.T.reshape(1, KC * KC)  # [a=j(power), b=k(cheb)] row-major

    def bf16_bytes(x):
        x = np.ascontiguousarray(np.asarray(x, np.float32))
        u32 = x.view(np.uint32)
        r = ((u32 + 0x7FFF + ((u32 >> 16) & 1)) >> 16).astype(np.uint16)
        return r

    cst = np.concatenate([
        np.tile(mu[None, :], (NP, 1)),
        np.tile(-beta[None, :], (NP, 1)),
        np.tile(linfrac[None, :], (NP, 1)),
        np.tile(Wm.reshape(1, KC * KC).astype(f), (NP, 1)),
        np.tile(t2m_packed.astype(f), (NP, 1)),
        np.eye(NP, dtype=f),
    ], axis=1).astype(f)

    stc = np.zeros((NP, NP), f)
    for k in range(NP):
        stc[k, k] = BIG
        stc[:k, k] = -BIG
    stall = np.full((NP, NP), -BIG, f)
    stone = np.ones((NP, 1), f)
    cstb = np.concatenate([bf16_bytes(stc), bf16_bytes(stall),
                           bf16_bytes(stone)], axis=1)

    pk = np.concatenate([
        aemb, tds[:, None], ddom[:, None], fourd[:, None], dtb[:, None],
        raw0[:, E - 1:E],
    ], axis=1).astype(f)

    in_maps = []
    for core in range(N_CORES):
        rs = slice(core * P, (core + 1) * P)
        uT = np.ascontiguousarray(us[rs].transpose(2, 1, 0))   # [E, S, P]
        rwt = bf16_bytes(raws[rs].T)                           # [E, P] bf16
        in_maps.append(dict(
            pk=np.ascontiguousarray(pk[rs]),
            cst=cst,
            cstb=cstb,
            rws=np.ascontiguousarray(raws[rs]),
            rwt=np.ascontiguousarray(rwt),
            ut=uT,
        ))
    return in_maps


def kernel(time_seq, time_delta_seq, event_seq, dtime_boundary, exp_raw,
           unif_numbers, mu, alpha, beta, type_emb, _trace=False):
    if "nc" not in _CACHE:
        _CACHE["nc"] = build_program()
    nc = _CACHE["nc"]

    in_maps = _prep_inputs(time_seq, time_delta_seq, event_seq, dtime_boundary,
                           exp_raw, unif_numbers, mu, alpha, beta, type_emb)

    out = run_bass_kernel_spmd(nc, in_maps, core_ids=list(range(N_CORES)),
                               trace=_trace)
    _CACHE["last_results"] = out

    res = np.concatenate([out.results[c]["res"].reshape(BPC, L, S)
                          for c in range(N_CORES)], axis=0)
    weights = np.full((B, L, S), np.float32(1.0 / S), np.float32)
    return res, weights


# revision 43
# speedup vs baseline: 1.0698x; 1.0095x over previous
"""Trainium2 Bass kernel for nn_EventSampler (thinning / rejection sampling).

kernel(**inputs) takes FULL unsharded inputs, shards batch across 8 cores
(2 batches = 256 (b,l) pairs per core), runs one SPMD Bass program, returns
the full output.

v3 structure (cost-model driven; per core):
  host prep: e-axis of (exp_raw, unif_numbers) sorted ascending by exp_raw
  per (b,l) pair, u transposed to [E, S, P] so the accept test runs in an
  e-on-partition layout.

  phase 1 (pair-layout, f32, same operation classes as the validated
  baseline): 25-point grid (20 bound-scan + 5 Chebyshev-Lobatto nodes) ->
  bounds -> b15/invb -> degree-4 polynomial tot(x) at the sorted draws ->
  q = tot*invb.  q is transposed to e-layout via PE transpose.

  phase 2 (e-layout, 16 pipelined pieces = 2 e-chunks x 8 4-row s-slices):
    m = (u < q) on DVE (the only full-size f32 pass),
    first-accept extraction on PE: counts = (200*I - 200*strict_tri) @ m
    (+ cross-chunk -200*ones @ m0; 512-col matmul halves, bank-aligned so
    start=True resets don't clobber the sibling half),
    IND = Act(Exp, counts - 200) in {0,1} exactly,
    fa = IND * raw_sorted (bf16; rows split FA_DVE_ROWS DVE, rest GpSimd),
    val = ones @ fa = raw* (or exact 0 if no accept), accumulated over both
    e-chunks in PSUM, parked to SBUF via Act copy.
  val rows return to pair-layout via PE transpose; tail (who = val>0,
  acc = val*invb, fallback max(last_raw*invb, dtb), min 1e5) is tiny.

Decision-critical arithmetic stays f32; bf16 only on the value path
(res tolerance 2e-2; host-sim on the real inputs shows max rel 3.9e-3).
Cost-model timeline: 62384 ns vs 80379 ns for the previous kernel.
"""

import os
import sys

import numpy as np

for _p in ("/opt/trn_rl_repo",):
    if _p not in sys.path and os.path.isdir(_p):
        sys.path.insert(0, _p)

import concourse.bacc as bacc
import concourse.tile as tile
import concourse.mybir as mybir
from concourse.bass_utils import run_bass_kernel_spmd

F32 = mybir.dt.float32
BF16 = mybir.dt.bfloat16

B, L, M = 16, 128, 32
S, E, S0 = 32, 256, 20
OVER = 1.5
KC = 5
G = S0 + KC
N_CORES = 8
BPC = B // N_CORES
P = BPC * L                     # pairs per core = 256
NP = 128
NCH = P // NP                   # pair chunks = 2
ECH = E // NP                   # e chunks = 2
RS = 4                          # s-rows per phase-2 piece
NSP = S // RS                   # s-pieces = 8
NPC = RS * P                    # free cols per piece = 1024
BIG = 200.0

# pk layout: aemb | tds | ddom | fourd | dtb | lastraw
WPK = M + 5
# cst (f32): mu | negbeta | linfrac | wfull(25) | t2m(25) | identity(128)
WCS = M + M + G + KC * KC + KC * KC + NP
# cstb (bf16): stc(128) | stall(128) | stone(1)
WCB = NP + NP + 1

FA_DVE_ROWS = 3                 # fa rows on DVE per piece (rest on GpSimd)

_CACHE = {}


def build_program():
    nc = bacc.Bacc("TRN2", target_bir_lowering=False, debug=False,
                   enable_asserts=False, num_devices=N_CORES)

    pk_d = nc.dram_tensor("pk", [P, WPK], F32, kind="ExternalInput")
    cst_d = nc.dram_tensor("cst", [NP, WCS], F32, kind="ExternalInput")
    cstb_d = nc.dram_tensor("cstb", [NP, WCB], BF16, kind="ExternalInput")
    rws_d = nc.dram_tensor("rws", [P, E], F32, kind="ExternalInput")
    rwt_d = nc.dram_tensor("rwt", [E, P], BF16, kind="ExternalInput")
    ut_d = nc.dram_tensor("ut", [E, S, P], F32, kind="ExternalInput")
    res_d = nc.dram_tensor("res", [P, S], F32, kind="ExternalOutput")
    DBG = os.environ.get("K_DBG") == "1"
    if DBG:
        mdbg_d = nc.dram_tensor("mdbg", [NP, RS, P], F32, kind="ExternalOutput")
        cdbg_d = nc.dram_tensor("cdbg", [NP, RS * P], F32, kind="ExternalOutput")
        idbg_d = nc.dram_tensor("idbg", [NP, RS, P], F32, kind="ExternalOutput")
        vdbg_d = nc.dram_tensor("vdbg", [1, RS * P], F32, kind="ExternalOutput")
        wdbg_d = nc.dram_tensor("wdbg", [S, P], F32, kind="ExternalOutput")

    alu = mybir.AluOpType
    act = mybir.ActivationFunctionType

    with tile.TileContext(nc) as tc:
        with (
            tc.tile_pool(name="io", bufs=1) as iop,
            tc.tile_pool(name="ubuf", bufs=1) as ubuf,
            tc.tile_pool(name="mbuf", bufs=1) as mbuf,
            tc.tile_pool(name="ph1", bufs=2) as ph1,
            tc.tile_pool(name="p2", bufs=4) as p2,
            tc.tile_pool(name="dbgp", bufs=1) as dbgp,
            tc.tile_pool(name="psA", bufs=2, space="PSUM") as psa,
            tc.tile_pool(name="psV", bufs=2, space="PSUM") as psv,
        ):
            # ---- DMAs: small packed inputs first, then the u stream ----
            pk_t = iop.tile([NP, NCH, WPK], F32, tag="pk")
            nc.sync.dma_start(out=pk_t[:],
                              in_=pk_d.ap().rearrange("(c p) w -> p c w", p=NP))
            cst_t = iop.tile([NP, WCS], F32, tag="cst")
            nc.sync.dma_start(out=cst_t[:], in_=cst_d.ap())
            cstb_t = iop.tile([NP, WCB], BF16, tag="cstb")
            nc.sync.dma_start(out=cstb_t[:], in_=cstb_d.ap())
            rws_t = iop.tile([NP, NCH, E], F32, tag="rws")
            nc.sync.dma_start(out=rws_t[:],
                              in_=rws_d.ap().rearrange("(c p) e -> p c e", p=NP))
            rwt_t = iop.tile([NP, ECH, P], BF16, tag="rwt")
            nc.sync.dma_start(out=rwt_t[:],
                              in_=rwt_d.ap().rearrange("(c a) p -> a c p", a=NP))

            ut = [ubuf.tile([NP, S, P], F32, tag=f"ut{a}", name=f"ut{a}")
                  for a in range(ECH)]
            DRS = 4
            for j in range(S // DRS):
                for a in range(ECH):
                    sl = slice(j * DRS, (j + 1) * DRS)
                    nc.sync.dma_start(
                        out=ut[a][:, sl, :],
                        in_=ut_d.ap().rearrange("(c a) s p -> a c s p", a=NP)
                            [:, a, sl, :])

            mu_t = cst_t[:, 0:M]
            negb_t = cst_t[:, M:2 * M]
            linfrac_t = cst_t[:, 2 * M:2 * M + G]
            o = 2 * M + G
            wfull_t = cst_t[:, o:o + KC * KC].rearrange("p (a b) -> p a b", a=KC)
            t2m_t = cst_t[:, o + KC * KC:o + 2 * KC * KC].rearrange(
                "p (a b) -> p a b", a=KC)
            ident_t = cst_t[:, o + 2 * KC * KC:]
            stc_t = cstb_t[:, 0:NP]
            stall_t = cstb_t[:, NP:2 * NP]
            stone_t = cstb_t[:, 2 * NP:2 * NP + 1]

            biasm = iop.tile([NP, 1], F32, tag="biasm")
            nc.gpsimd.memset(biasm[:], -BIG)

            negE = iop.tile([NP, G, M], F32, tag="negE")
            nc.vector.tensor_tensor(
                out=negE[:],
                in0=linfrac_t.unsqueeze(2).to_broadcast((NP, G, M)),
                in1=negb_t.unsqueeze(1).to_broadcast((NP, G, M)), op=alu.mult)

            # qT[a]: [e-part, pair(c major)] via PE transpose + copy to SBUF
            qT = iop.tile([NP, ECH, NP * NCH], F32, tag="qT")

            # both chunks' grids in single wide ops (fewer critical-path hops)
            dG2 = ph1.tile([NP, NCH, G, M], F32, tag="gA", name="dG2")
            for c in range(NCH):
                nc.scalar.activation(dG2[:, c, 0:S0, :], negE[:, 0:S0, :],
                                     act.Exp, scale=pk_t[:, c, M:M + 1])
                nc.scalar.activation(dG2[:, c, S0:G, :], negE[:, S0:G, :],
                                     act.Exp, scale=pk_t[:, c, M + 1:M + 2])
            gG2 = ph1.tile([NP, NCH, G, M], F32, tag="gB", name="gG2")
            nc.vector.tensor_tensor(
                out=gG2[:], in0=dG2[:],
                in1=pk_t[:, :, 0:M].unsqueeze(2).to_broadcast((NP, NCH, G, M)),
                op=alu.mult)
            sG2 = ph1.tile([NP, NCH, G, M], F32, tag="gA", name="sG2")
            nc.vector.tensor_tensor(
                out=sG2[:], in0=gG2[:],
                in1=mu_t.unsqueeze(1).unsqueeze(1).to_broadcast((NP, NCH, G, M)),
                op=alu.add)
            eG2 = ph1.tile([NP, NCH, G, M], F32, tag="gB", name="eG2")
            nc.scalar.activation(eG2[:], sG2[:], act.Exp)
            spG2 = ph1.tile([NP, NCH, G, M], F32, tag="gA", name="spG2")
            nc.scalar.activation(spG2[:], eG2[:], act.Ln, bias=1.0)
            vals2 = ph1.tile([NP, NCH, G], F32, tag="vals", name="vals2")
            nc.vector.reduce_sum(out=vals2[:], in_=spG2[:],
                                 axis=mybir.AxisListType.X)

            ch = [dict() for _ in range(NCH)]
            for c in range(NCH):
                d = ch[c]
                aemb = pk_t[:, c, 0:M]
                tds = pk_t[:, c, M:M + 1]
                ddom = pk_t[:, c, M + 1:M + 2]
                fourd = pk_t[:, c, M + 2:M + 3]
                d["dtb"] = pk_t[:, c, M + 3:M + 4]
                d["lastraw"] = pk_t[:, c, M + 4:M + 5]
                raw = rws_t[:, c, :]
                vals = type("V", (), {})()  # view shim
                vals = None

                bmax = ph1.tile([NP, 1], F32, tag="bmax", name=f"bmax{c}")
                nc.vector.reduce_max(out=bmax[:], in_=vals2[:, c, 0:S0],
                                     axis=mybir.AxisListType.X)
                b15 = ph1.tile([NP, 1], F32, tag="b15", name=f"b15{c}")
                nc.vector.tensor_scalar(out=b15[:], in0=bmax[:],
                                        scalar1=float(OVER), scalar2=None,
                                        op0=alu.mult)
                invb = ph1.tile([NP, 1], F32, tag="invb", name=f"invb{c}")
                nc.vector.reciprocal(invb[:], b15[:])
                svc2 = ph1.tile([NP, 1], F32, tag="svc2", name=f"svc2{c}")
                nc.vector.tensor_scalar(out=svc2[:], in0=invb[:], scalar1=fourd,
                                        scalar2=None, op0=alu.mult)
                w2 = ph1.tile([NP, E], F32, tag="w2", name=f"w2{c}")
                nc.vector.tensor_scalar(out=w2[:], in0=raw, scalar1=svc2[:],
                                        scalar2=-2.0, op0=alu.mult, op1=alu.add)

                cw = ph1.tile([NP, KC, KC], F32, tag="cw", name=f"cw{c}")
                nc.vector.tensor_tensor(
                    out=cw[:],
                    in0=vals2[:, c, S0:G].unsqueeze(1).to_broadcast((NP, KC, KC)),
                    in1=wfull_t, op=alu.mult)
                cc = ph1.tile([NP, KC], F32, tag="cc", name=f"cc{c}")
                nc.vector.reduce_sum(out=cc[:], in_=cw[:],
                                     axis=mybir.AxisListType.X)
                cw2 = ph1.tile([NP, KC, KC], F32, tag="cw2", name=f"cw2{c}")
                nc.vector.tensor_tensor(
                    out=cw2[:],
                    in0=cc[:].unsqueeze(1).to_broadcast((NP, KC, KC)),
                    in1=t2m_t, op=alu.mult)
                am = ph1.tile([NP, KC], F32, tag="am", name=f"am{c}")
                nc.vector.reduce_sum(out=am[:], in_=cw2[:],
                                     axis=mybir.AxisListType.X)

                x2 = ph1.tile([NP, E], F32, tag="x2", name=f"x2{c}")
                nc.vector.tensor_tensor(out=x2[:], in0=w2[:], in1=w2[:],
                                        op=alu.mult)
                u1 = ph1.tile([NP, E], F32, tag="u1", name=f"u1{c}")
                nc.vector.tensor_scalar(out=u1[:], in0=x2[:],
                                        scalar1=am[:, 4:5], scalar2=am[:, 2:3],
                                        op0=alu.mult, op1=alu.add)
                u2 = ph1.tile([NP, E], F32, tag="u2", name=f"u2{c}")
                nc.vector.tensor_tensor(out=u2[:], in0=u1[:], in1=x2[:],
                                        op=alu.mult)
                v1 = ph1.tile([NP, E], F32, tag="v1", name=f"v1{c}")
                nc.vector.tensor_scalar(out=v1[:], in0=x2[:],
                                        scalar1=am[:, 3:4], scalar2=am[:, 1:2],
                                        op0=alu.mult, op1=alu.add)
                v2 = ph1.tile([NP, E], F32, tag="v2", name=f"v2{c}")
                nc.vector.tensor_tensor(out=v2[:], in0=v1[:], in1=w2[:],
                                        op=alu.mult)
                tot = ph1.tile([NP, E], F32, tag="tot", name=f"tot{c}")
                nc.vector.scalar_tensor_tensor(out=tot[:], in0=u2[:],
                                               scalar=am[:, 0:1], in1=v2[:],
                                               op0=alu.add, op1=alu.add)
                q = ph1.tile([NP, E], F32, tag="q", name=f"q{c}")
                nc.vector.tensor_scalar(out=q[:], in0=tot[:], scalar1=invb[:],
                                        scalar2=None, op0=alu.mult)
                # q [pair, e] -> qT [e, pair] (PE transpose + SBUF copy)
                for a in range(ECH):
                    qsc = psa.tile([NP, NPC], F32, tag="cnt", name=f"qsc{c}_{a}")
                    nc.tensor.transpose(qsc[:, 0:NP],
                                        q[:, a * NP:(a + 1) * NP], ident_t)
                    nc.scalar.activation(qT[:, a, c * NP:(c + 1) * NP],
                                          qsc[:, 0:NP], act.Copy)
                d.update(invb=invb)

            # ---- phase 2: 16 pieces ----
            vst = iop.tile([S, P], F32, tag="vst")
            m_t = [mbuf.tile([NP, S, P], BF16, tag=f"m{a}", name=f"m{a}")
                   for a in range(ECH)]
            for j in range(NSP):
                sl = slice(j * RS, (j + 1) * RS)
                for a in range(ECH):
                    nc.vector.tensor_tensor(
                        out=m_t[a][:, sl, :], in0=ut[a][:, sl, :],
                        in1=qT[:, a, :].unsqueeze(1).to_broadcast((NP, RS, P)),
                        op=alu.is_lt)
                    mv = m_t[a][:, sl, :].rearrange("a s p -> a (s p)")
                    cnt = psa.tile([NP, NPC], F32, tag="cnt", name=f"cnt{a}_{j}")
                    HH = NPC // 2
                    for h in range(2):
                        hs = slice(h * HH, (h + 1) * HH)
                        if a == 0:
                            nc.tensor.matmul(cnt[:, hs], stc_t, mv[:, hs],
                                             start=True, stop=True)
                        else:
                            nc.tensor.matmul(cnt[:, hs], stc_t, mv[:, hs],
                                             start=True, stop=False)
                            nc.tensor.matmul(
                                cnt[:, hs], stall_t,
                                m_t[0][:, sl, :].rearrange("a s p -> a (s p)")[:, hs],
                                start=False, stop=True)
                    ind = p2.tile([NP, RS, P], BF16, tag="ind", name=f"ind{a}_{j}")
                    nc.scalar.activation(ind[:].rearrange("a s p -> a (s p)"),
                                         cnt[:], act.Exp, bias=biasm[:])
                    rbc = rwt_t[:, a, :].unsqueeze(1)
                    fd = FA_DVE_ROWS
                    fa = p2.tile([NP, RS, P], BF16, tag="fa", name=f"fa{a}_{j}")
                    nc.vector.tensor_tensor(
                        out=fa[:, 0:fd, :], in0=ind[:, 0:fd, :],
                        in1=rbc.to_broadcast((NP, fd, P)), op=alu.mult)
                    nc.gpsimd.tensor_tensor(
                        out=fa[:, fd:RS, :], in0=ind[:, fd:RS, :],
                        in1=rbc.to_broadcast((NP, RS - fd, P)), op=alu.mult)
                    if a == 0:
                        pv = psv.tile([1, NPC], F32, tag="pv", name=f"pv{j}")
                        pv_hold = pv
                    else:
                        pv = pv_hold
                    fav = fa[:].rearrange("a s p -> a (s p)")
                    for h in range(2):
                        hs = slice(h * (NPC // 2), (h + 1) * (NPC // 2))
                        nc.tensor.matmul(pv[:, hs], stone_t, fav[:, hs],
                                         start=(a == 0), stop=(a == 1))
                    if DBG and j == 1 and a == 0:
                        t1 = dbgp.tile([NP, RS, P], F32, tag="dbg1")
                        nc.vector.tensor_copy(t1[:], m_t[a][:, sl, :])
                        nc.sync.dma_start(out=mdbg_d.ap(), in_=t1[:])
                        t2 = dbgp.tile([NP, RS * P], F32, tag="dbg2")
                        nc.vector.tensor_copy(t2[:], cnt[:])
                        nc.sync.dma_start(out=cdbg_d.ap(), in_=t2[:])
                        t3 = dbgp.tile([NP, RS, P], F32, tag="dbg3")
                        nc.vector.tensor_copy(t3[:], ind[:])
                        nc.sync.dma_start(out=idbg_d.ap(), in_=t3[:])
                    if DBG and j == 1 and a == 1:
                        t4 = dbgp.tile([1, RS * P], F32, tag="dbg4")
                        nc.vector.tensor_copy(t4[:], pv[:])
                        nc.sync.dma_start(out=vdbg_d.ap(), in_=t4[:])
                    if a == 1:
                        svj = p2.tile([1, NPC], F32, tag=f"sv{j % 2}",
                                      name=f"sv{j}")
                        nc.scalar.activation(svj[:], pv[:], act.Copy)
                        nc.sync.dma_start(
                            out=vst[j * RS:(j + 1) * RS, :],
                            in_=svj[:].rearrange("o (s p) -> o s p", s=RS))

            if DBG:
                nc.sync.dma_start(out=wdbg_d.ap(), in_=vst[:])
            # PE-transpose the collected val rows back to pair-layout
            valT = psa.tile([NP, NPC], F32, tag="cnt", name="valT")
            for c in range(NCH):
                nc.tensor.transpose(valT[:, c * S:(c + 1) * S],
                                    vst[:, c * NP:(c + 1) * NP],
                                    ident_t[0:S, 0:S])

            resall = ph1.tile([NP, NCH, S], F32, tag="resall")
            whoall = ph1.tile([NP, NCH * S], mybir.dt.int32, tag="whoall")
            nc.vector.tensor_scalar(out=whoall[:], in0=valT[:, 0:NCH * S],
                                    scalar1=0.0, scalar2=None, op0=alu.is_gt)
            for c in range(NCH):
                d = ch[c]
                invb = d["invb"]
                val = valT[:, c * S:(c + 1) * S]
                acc = ph1.tile([NP, S], F32, tag="acc", name=f"acc{c}")
                nc.vector.tensor_scalar(out=acc[:], in0=val, scalar1=invb[:],
                                        scalar2=None, op0=alu.mult)
                lastx = ph1.tile([NP, 1], F32, tag="lastx", name=f"lastx{c}")
                nc.vector.tensor_scalar(out=lastx[:], in0=d["lastraw"],
                                        scalar1=invb[:], scalar2=None,
                                        op0=alu.mult)
                fb = ph1.tile([NP, 1], F32, tag="fb", name=f"fb{c}")
                nc.vector.tensor_tensor(out=fb[:], in0=lastx[:], in1=d["dtb"],
                                        op=alu.max)
                nc.scalar.activation(resall[:, c, :],
                                     fb[:].to_broadcast((NP, S)), act.Copy)
                nc.vector.copy_predicated(resall[:, c, :],
                                          whoall[:, c * S:(c + 1) * S], acc[:])
            res2_t = ph1.tile([NP, NCH, S], F32, tag="res2")
            nc.vector.tensor_scalar(out=res2_t[:], in0=resall[:], scalar1=1e5,
                                    scalar2=None, op0=alu.min)
            nc.sync.dma_start(
                out=res_d.ap().rearrange("(c p) s -> p c s", p=NP),
                in_=res2_t[:])

    nc.finalize()
    return nc


def _prep_inputs(time_seq, time_delta_seq, event_seq, dtime_boundary, exp_raw,
                 unif_numbers, mu, alpha, beta, type_emb):
    f = np.float32
    tds = np.asarray(time_delta_seq, f).reshape(B * L)
    dtb = np.asarray(dtime_boundary, f).reshape(B * L)
    raw0 = np.asarray(exp_raw, f).reshape(B * L, E)
    u = np.asarray(unif_numbers, f).reshape(B * L, S, E)
    ev = np.asarray(event_seq)
    mu = np.asarray(mu, f)
    alpha = np.asarray(alpha, f)
    beta = np.asarray(beta, f)
    type_emb = np.asarray(type_emb, f)

    aemb = (alpha[None, :] * type_emb)[ev].reshape(B * L, M).astype(f)

    order = np.argsort(raw0, axis=-1, kind="stable")
    raws = np.take_along_axis(raw0, order, axis=-1).astype(f)
    us = np.take_along_axis(u, order[:, None, :], axis=-1).astype(f)

    tot00 = np.log1p(np.exp((aemb + mu[None, :]).astype(np.float64))).sum(-1)
    rawmax = raw0.max(-1).astype(np.float64)
    Ddom = rawmax / (1.5 * tot00)
    fourd = (4.0 / Ddom).astype(f)
    ddom = Ddom.astype(f)

    jj = np.arange(KC)
    n = KC - 1
    frac = (1.0 + np.cos(np.pi * jj / n)) / 2.0
    linfrac = np.concatenate([np.linspace(0.0, 1.0, S0), frac]).astype(f)

    Wm = np.zeros((KC, KC))
    for k in range(KC):
        wrow = np.cos(np.pi * jj * k / n)
        wrow[0] *= 0.5
        wrow[-1] *= 0.5
        wrow *= 2.0 / n
        if k == 0 or k == n:
            wrow *= 0.5
        Wm[k] = wrow
    # tot = sum_k cc_k T_k(w2/2); T_k(w2/2) as powers of w2 (cols = power)
    t2m = np.zeros((KC, KC))
    t2m[0, 0] = 1.0
    t2m[1, 1] = 0.5
    t2m[2, 0], t2m[2, 2] = -1.0, 0.5
    t2m[3, 1], t2m[3, 3] = -1.5, 0.5
    t2m[4, 0], t2m[4, 2], t2m[4, 4] = 1.0, -2.0, 0.5
    # cw2 uses cc broadcast over rows a: am_j = sum_b cc_b * t2m[b, j]
    t2m_packed = t2m.T.reshape(1, KC * KC)  # [a=j(power), b=k(cheb)] row-major

    def bf16_bytes(x):
        x = np.ascontiguousarray(np.asarray(x, np.float32))
        u32 = x.view(np.uint32)
        r = ((u32 + 0x7FFF + ((u32 >> 16) & 1)) >> 16).astype(np.uint16)
        return r

    cst = np.concatenate([
        np.tile(mu[None, :], (NP, 1)),
        np.tile(-beta[None, :], (NP, 1)),
        np.tile(linfrac[None, :], (NP, 1)),
        np.tile(Wm.reshape(1, KC * KC).astype(f), (NP, 1)),
        np.tile(t2m_packed.astype(f), (NP, 1)),
        np.eye(NP, dtype=f),
    ], axis=1).astype(f)

    stc = np.zeros((NP, NP), f)
    for k in range(NP):
        stc[k, k] = BIG
        stc[:k, k] = -BIG
    stall = np.full((NP, NP), -BIG, f)
    stone = np.ones((NP, 1), f)
    cstb = np.concatenate([bf16_bytes(stc), bf16_bytes(stall),
                           bf16_bytes(stone)], axis=1)

    pk = np.concatenate([
        aemb, tds[:, None], ddom[:, None], fourd[:, None], dtb[:, None],
        raw0[:, E - 1:E],
    ], axis=1).astype(f)

    in_maps = []
    for core in range(N_CORES):
        rs = slice(core * P, (core + 1) * P)
        uT = np.ascontiguousarray(us[rs].transpose(2, 1, 0))   # [E, S, P]
        rwt = bf16_bytes(raws[rs].T)                           # [E, P] bf16
        in_maps.append(dict(
            pk=np.ascontiguousarray(pk[rs]),
            cst=cst,
            cstb=cstb,
            rws=np.ascontiguousarray(raws[rs]),
            rwt=np.ascontiguousarray(rwt),
            ut=uT,
        ))
    return in_maps


def kernel(time_seq, time_delta_seq, event_seq, dtime_boundary, exp_raw,
           unif_numbers, mu, alpha, beta, type_emb, _trace=False):
    if "nc" not in _CACHE:
        _CACHE["nc"] = build_program()
    nc = _CACHE["nc"]

    in_maps = _prep_inputs(time_seq, time_delta_seq, event_seq, dtime_boundary,
                           exp_raw, unif_numbers, mu, alpha, beta, type_emb)

    out = run_bass_kernel_spmd(nc, in_maps, core_ids=list(range(N_CORES)),
                               trace=_trace)
    _CACHE["last_results"] = out

    res = np.concatenate([out.results[c]["res"].reshape(BPC, L, S)
                          for c in range(N_CORES)], axis=0)
    weights = np.full((B, L, S), np.float32(1.0 / S), np.float32)
    return res, weights


# revision 45
# speedup vs baseline: 1.0723x; 1.0023x over previous
"""Trainium2 Bass kernel for nn_EventSampler (thinning / rejection sampling).

kernel(**inputs) takes FULL unsharded inputs, shards batch across 8 cores
(2 batches = 256 (b,l) pairs per core), runs one SPMD Bass program, returns
the full output.

v3 structure (cost-model driven; per core):
  host prep: e-axis of (exp_raw, unif_numbers) sorted ascending by exp_raw
  per (b,l) pair, u transposed to [E, S, P] so the accept test runs in an
  e-on-partition layout.

  phase 1 (pair-layout, f32, same operation classes as the validated
  baseline): 25-point grid (20 bound-scan + 5 Chebyshev-Lobatto nodes) ->
  bounds -> b15/invb -> degree-4 polynomial tot(x) at the sorted draws ->
  q = tot*invb.  q is transposed to e-layout via PE transpose.

  phase 2 (e-layout, 16 pipelined pieces = 2 e-chunks x 8 4-row s-slices):
    m = (u < q) on DVE (the only full-size f32 pass),
    first-accept extraction on PE: counts = (200*I - 200*strict_tri) @ m
    (+ cross-chunk -200*ones @ m0; 512-col matmul halves, bank-aligned so
    start=True resets don't clobber the sibling half),
    IND = Act(Exp, counts - 200) in {0,1} exactly,
    fa = IND * raw_sorted (bf16; rows split FA_DVE_ROWS DVE, rest GpSimd),
    val = ones @ fa = raw* (or exact 0 if no accept), accumulated over both
    e-chunks in PSUM, parked to SBUF via Act copy.
  val rows return to pair-layout via PE transpose; tail (who = val>0,
  acc = val*invb, fallback max(last_raw*invb, dtb), min 1e5) is tiny.

Decision-critical arithmetic stays f32; bf16 only on the value path
(res tolerance 2e-2; host-sim on the real inputs shows max rel 3.9e-3).
Cost-model timeline: 62384 ns vs 80379 ns for the previous kernel.
"""

import os
import sys

import numpy as np

for _p in ("/opt/trn_rl_repo",):
    if _p not in sys.path and os.path.isdir(_p):
        sys.path.insert(0, _p)

import concourse.bacc as bacc
import concourse.tile as tile
import concourse.mybir as mybir
from concourse.bass_utils import run_bass_kernel_spmd

F32 = mybir.dt.float32
BF16 = mybir.dt.bfloat16

B, L, M = 16, 128, 32
S, E, S0 = 32, 256, 20
OVER = 1.5
KC = 5
G = S0 + KC
N_CORES = 8
BPC = B // N_CORES
P = BPC * L                     # pairs per core = 256
NP = 128
NCH = P // NP                   # pair chunks = 2
ECH = E // NP                   # e chunks = 2
RS = 4                          # s-rows per phase-2 piece
NSP = S // RS                   # s-pieces = 8
NPC = RS * P                    # free cols per piece = 1024
BIG = 200.0

# pk layout: aemb | tds | ddom | fourd | dtb | lastraw
WPK = M + 5
# cst (f32): mu | negbeta | linfrac | wfull(25) | t2m(25) | identity(128)
WCS = M + M + G + KC * KC + KC * KC + NP
# cstb (bf16): stc(128) | stall(128) | stone(1)
WCB = NP + NP + 1

FA_DVE_ROWS = 3                 # fa rows on DVE per piece (rest on GpSimd)

_CACHE = {}


def build_program():
    nc = bacc.Bacc("TRN2", target_bir_lowering=False, debug=False,
                   enable_asserts=False, num_devices=N_CORES)

    pk_d = nc.dram_tensor("pk", [P, WPK], F32, kind="ExternalInput")
    cst_d = nc.dram_tensor("cst", [NP, WCS], F32, kind="ExternalInput")
    cstb_d = nc.dram_tensor("cstb", [NP, WCB], BF16, kind="ExternalInput")
    rws_d = nc.dram_tensor("rws", [P, E], F32, kind="ExternalInput")
    rwt_d = nc.dram_tensor("rwt", [E, P], BF16, kind="ExternalInput")
    ut_d = nc.dram_tensor("ut", [E, S, P], F32, kind="ExternalInput")
    res_d = nc.dram_tensor("res", [P, S], F32, kind="ExternalOutput")
    DBG = os.environ.get("K_DBG") == "1"
    if DBG:
        mdbg_d = nc.dram_tensor("mdbg", [NP, RS, P], F32, kind="ExternalOutput")
        cdbg_d = nc.dram_tensor("cdbg", [NP, RS * P], F32, kind="ExternalOutput")
        idbg_d = nc.dram_tensor("idbg", [NP, RS, P], F32, kind="ExternalOutput")
        vdbg_d = nc.dram_tensor("vdbg", [1, RS * P], F32, kind="ExternalOutput")
        wdbg_d = nc.dram_tensor("wdbg", [S, P], F32, kind="ExternalOutput")

    alu = mybir.AluOpType
    act = mybir.ActivationFunctionType

    with tile.TileContext(nc) as tc:
        with (
            tc.tile_pool(name="io", bufs=1) as iop,
            tc.tile_pool(name="ubuf", bufs=1) as ubuf,
            tc.tile_pool(name="mbuf", bufs=1) as mbuf,
            tc.tile_pool(name="ph1", bufs=2) as ph1,
            tc.tile_pool(name="p2", bufs=4) as p2,
            tc.tile_pool(name="dbgp", bufs=1) as dbgp,
            tc.tile_pool(name="psA", bufs=3, space="PSUM") as psa,
            tc.tile_pool(name="psV", bufs=1, space="PSUM") as psv,
        ):
            # ---- DMAs: small packed inputs first, then the u stream ----
            pk_t = iop.tile([NP, NCH, WPK], F32, tag="pk")
            nc.sync.dma_start(out=pk_t[:],
                              in_=pk_d.ap().rearrange("(c p) w -> p c w", p=NP))
            cst_t = iop.tile([NP, WCS], F32, tag="cst")
            nc.sync.dma_start(out=cst_t[:], in_=cst_d.ap())
            cstb_t = iop.tile([NP, WCB], BF16, tag="cstb")
            nc.sync.dma_start(out=cstb_t[:], in_=cstb_d.ap())
            rws_t = iop.tile([NP, NCH, E], F32, tag="rws")
            nc.sync.dma_start(out=rws_t[:],
                              in_=rws_d.ap().rearrange("(c p) e -> p c e", p=NP))
            rwt_t = iop.tile([NP, ECH, P], BF16, tag="rwt")
            nc.sync.dma_start(out=rwt_t[:],
                              in_=rwt_d.ap().rearrange("(c a) p -> a c p", a=NP))

            ut = [ubuf.tile([NP, S, P], F32, tag=f"ut{a}", name=f"ut{a}")
                  for a in range(ECH)]
            DRS = 4
            for j in range(S // DRS):
                for a in range(ECH):
                    sl = slice(j * DRS, (j + 1) * DRS)
                    nc.sync.dma_start(
                        out=ut[a][:, sl, :],
                        in_=ut_d.ap().rearrange("(c a) s p -> a c s p", a=NP)
                            [:, a, sl, :])

            mu_t = cst_t[:, 0:M]
            negb_t = cst_t[:, M:2 * M]
            linfrac_t = cst_t[:, 2 * M:2 * M + G]
            o = 2 * M + G
            wfull_t = cst_t[:, o:o + KC * KC].rearrange("p (a b) -> p a b", a=KC)
            t2m_t = cst_t[:, o + KC * KC:o + 2 * KC * KC].rearrange(
                "p (a b) -> p a b", a=KC)
            ident_t = cst_t[:, o + 2 * KC * KC:]
            stc_t = cstb_t[:, 0:NP]
            stall_t = cstb_t[:, NP:2 * NP]
            stone_t = cstb_t[:, 2 * NP:2 * NP + 1]

            biasm = iop.tile([NP, 1], F32, tag="biasm")
            nc.gpsimd.memset(biasm[:], -BIG)

            negE = iop.tile([NP, G, M], F32, tag="negE")
            nc.vector.tensor_tensor(
                out=negE[:],
                in0=linfrac_t.unsqueeze(2).to_broadcast((NP, G, M)),
                in1=negb_t.unsqueeze(1).to_broadcast((NP, G, M)), op=alu.mult)

            # qT[a]: [e-part, pair(c major)] via PE transpose + copy to SBUF
            qT = iop.tile([NP, ECH, NP * NCH], F32, tag="qT")

            # both chunks' grids in single wide ops (fewer critical-path hops)
            dG2 = ph1.tile([NP, NCH, G, M], F32, tag="gA", name="dG2")
            for c in range(NCH):
                nc.scalar.activation(dG2[:, c, 0:S0, :], negE[:, 0:S0, :],
                                     act.Exp, scale=pk_t[:, c, M:M + 1])
                nc.scalar.activation(dG2[:, c, S0:G, :], negE[:, S0:G, :],
                                     act.Exp, scale=pk_t[:, c, M + 1:M + 2])
            gG2 = ph1.tile([NP, NCH, G, M], F32, tag="gB", name="gG2")
            nc.vector.tensor_tensor(
                out=gG2[:], in0=dG2[:],
                in1=pk_t[:, :, 0:M].unsqueeze(2).to_broadcast((NP, NCH, G, M)),
                op=alu.mult)
            sG2 = ph1.tile([NP, NCH, G, M], F32, tag="gA", name="sG2")
            nc.vector.tensor_tensor(
                out=sG2[:], in0=gG2[:],
                in1=mu_t.unsqueeze(1).unsqueeze(1).to_broadcast((NP, NCH, G, M)),
                op=alu.add)
            eG2 = ph1.tile([NP, NCH, G, M], F32, tag="gB", name="eG2")
            nc.scalar.activation(eG2[:], sG2[:], act.Exp)
            spG2 = ph1.tile([NP, NCH, G, M], F32, tag="gA", name="spG2")
            nc.scalar.activation(spG2[:], eG2[:], act.Ln, bias=1.0)
            vals2 = ph1.tile([NP, NCH, G], F32, tag="vals", name="vals2")
            nc.vector.reduce_sum(out=vals2[:], in_=spG2[:],
                                 axis=mybir.AxisListType.X)

            ch = [dict() for _ in range(NCH)]
            for c in range(NCH):
                d = ch[c]
                aemb = pk_t[:, c, 0:M]
                tds = pk_t[:, c, M:M + 1]
                ddom = pk_t[:, c, M + 1:M + 2]
                fourd = pk_t[:, c, M + 2:M + 3]
                d["dtb"] = pk_t[:, c, M + 3:M + 4]
                d["lastraw"] = pk_t[:, c, M + 4:M + 5]
                raw = rws_t[:, c, :]
                vals = type("V", (), {})()  # view shim
                vals = None

                bmax = ph1.tile([NP, 1], F32, tag="bmax", name=f"bmax{c}")
                nc.vector.reduce_max(out=bmax[:], in_=vals2[:, c, 0:S0],
                                     axis=mybir.AxisListType.X)
                b15 = ph1.tile([NP, 1], F32, tag="b15", name=f"b15{c}")
                nc.vector.tensor_scalar(out=b15[:], in0=bmax[:],
                                        scalar1=float(OVER), scalar2=None,
                                        op0=alu.mult)
                invb = ph1.tile([NP, 1], F32, tag="invb", name=f"invb{c}")
                nc.vector.reciprocal(invb[:], b15[:])
                svc2 = ph1.tile([NP, 1], F32, tag="svc2", name=f"svc2{c}")
                nc.vector.tensor_scalar(out=svc2[:], in0=invb[:], scalar1=fourd,
                                        scalar2=None, op0=alu.mult)
                w2 = ph1.tile([NP, E], F32, tag="w2", name=f"w2{c}")
                nc.vector.tensor_scalar(out=w2[:], in0=raw, scalar1=svc2[:],
                                        scalar2=-2.0, op0=alu.mult, op1=alu.add)

                cw = ph1.tile([NP, KC, KC], F32, tag="cw", name=f"cw{c}")
                nc.vector.tensor_tensor(
                    out=cw[:],
                    in0=vals2[:, c, S0:G].unsqueeze(1).to_broadcast((NP, KC, KC)),
                    in1=wfull_t, op=alu.mult)
                cc = ph1.tile([NP, KC], F32, tag="cc", name=f"cc{c}")
                nc.vector.reduce_sum(out=cc[:], in_=cw[:],
                                     axis=mybir.AxisListType.X)
                cw2 = ph1.tile([NP, KC, KC], F32, tag="cw2", name=f"cw2{c}")
                nc.vector.tensor_tensor(
                    out=cw2[:],
                    in0=cc[:].unsqueeze(1).to_broadcast((NP, KC, KC)),
                    in1=t2m_t, op=alu.mult)
                am = ph1.tile([NP, KC], F32, tag="am", name=f"am{c}")
                nc.vector.reduce_sum(out=am[:], in_=cw2[:],
                                     axis=mybir.AxisListType.X)

                x2 = ph1.tile([NP, E], F32, tag="x2", name=f"x2{c}")
                nc.vector.tensor_tensor(out=x2[:], in0=w2[:], in1=w2[:],
                                        op=alu.mult)
                u1 = ph1.tile([NP, E], F32, tag="u1", name=f"u1{c}")
                nc.vector.tensor_scalar(out=u1[:], in0=x2[:],
                                        scalar1=am[:, 4:5], scalar2=am[:, 2:3],
                                        op0=alu.mult, op1=alu.add)
                u2 = ph1.tile([NP, E], F32, tag="u2", name=f"u2{c}")
                nc.vector.tensor_tensor(out=u2[:], in0=u1[:], in1=x2[:],
                                        op=alu.mult)
                v1 = ph1.tile([NP, E], F32, tag="v1", name=f"v1{c}")
                nc.vector.tensor_scalar(out=v1[:], in0=x2[:],
                                        scalar1=am[:, 3:4], scalar2=am[:, 1:2],
                                        op0=alu.mult, op1=alu.add)
                v2 = ph1.tile([NP, E], F32, tag="v2", name=f"v2{c}")
                nc.vector.tensor_tensor(out=v2[:], in0=v1[:], in1=w2[:],
                                        op=alu.mult)
                tot = ph1.tile([NP, E], F32, tag="tot", name=f"tot{c}")
                nc.vector.scalar_tensor_tensor(out=tot[:], in0=u2[:],
                                               scalar=am[:, 0:1], in1=v2[:],
                                               op0=alu.add, op1=alu.add)
                q = ph1.tile([NP, E], F32, tag="q", name=f"q{c}")
                nc.vector.tensor_scalar(out=q[:], in0=tot[:], scalar1=invb[:],
                                        scalar2=None, op0=alu.mult)
                # q [pair, e] -> qT [e, pair] (PE transpose + SBUF copy)
                for a in range(ECH):
                    qsc = psa.tile([NP, NPC], F32, tag="cnt", name=f"qsc{c}_{a}")
                    nc.tensor.transpose(qsc[:, 0:NP],
                                        q[:, a * NP:(a + 1) * NP], ident_t)
                    nc.scalar.activation(qT[:, a, c * NP:(c + 1) * NP],
                                          qsc[:, 0:NP], act.Copy)
                d.update(invb=invb)

            # ---- phase 2: 16 pieces ----
            vst = iop.tile([S, P], F32, tag="vst")
            m_t = [mbuf.tile([NP, S, P], BF16, tag=f"m{a}", name=f"m{a}")
                   for a in range(ECH)]
            for j in range(NSP):
                sl = slice(j * RS, (j + 1) * RS)
                for a in range(ECH):
                    nc.vector.tensor_tensor(
                        out=m_t[a][:, sl, :], in0=ut[a][:, sl, :],
                        in1=qT[:, a, :].unsqueeze(1).to_broadcast((NP, RS, P)),
                        op=alu.is_lt)
                    mv = m_t[a][:, sl, :].rearrange("a s p -> a (s p)")
                    cnt = psa.tile([NP, NPC], F32, tag="cnt", name=f"cnt{a}_{j}")
                    HH = NPC // 2
                    for h in range(2):
                        hs = slice(h * HH, (h + 1) * HH)
                        if a == 0:
                            nc.tensor.matmul(cnt[:, hs], stc_t, mv[:, hs],
                                             start=True, stop=True)
                        else:
                            nc.tensor.matmul(cnt[:, hs], stc_t, mv[:, hs],
                                             start=True, stop=False)
                            nc.tensor.matmul(
                                cnt[:, hs], stall_t,
                                m_t[0][:, sl, :].rearrange("a s p -> a (s p)")[:, hs],
                                start=False, stop=True)
                    ind = p2.tile([NP, RS, P], BF16, tag="ind", name=f"ind{a}_{j}")
                    nc.scalar.activation(ind[:].rearrange("a s p -> a (s p)"),
                                         cnt[:], act.Exp, bias=biasm[:])
                    rbc = rwt_t[:, a, :].unsqueeze(1)
                    fd = FA_DVE_ROWS
                    fa = p2.tile([NP, RS, P], BF16, tag="fa", name=f"fa{a}_{j}")
                    nc.vector.tensor_tensor(
                        out=fa[:, 0:fd, :], in0=ind[:, 0:fd, :],
                        in1=rbc.to_broadcast((NP, fd, P)), op=alu.mult)
                    nc.gpsimd.tensor_tensor(
                        out=fa[:, fd:RS, :], in0=ind[:, fd:RS, :],
                        in1=rbc.to_broadcast((NP, RS - fd, P)), op=alu.mult)
                    if a == 0:
                        pv = psv.tile([1, NPC], F32, tag="pv", name=f"pv{j}")
                        pv_hold = pv
                    else:
                        pv = pv_hold
                    fav = fa[:].rearrange("a s p -> a (s p)")
                    for h in range(2):
                        hs = slice(h * (NPC // 2), (h + 1) * (NPC // 2))
                        nc.tensor.matmul(pv[:, hs], stone_t, fav[:, hs],
                                         start=(a == 0), stop=(a == 1))
                    if DBG and j == 1 and a == 0:
                        t1 = dbgp.tile([NP, RS, P], F32, tag="dbg1")
                        nc.vector.tensor_copy(t1[:], m_t[a][:, sl, :])
                        nc.sync.dma_start(out=mdbg_d.ap(), in_=t1[:])
                        t2 = dbgp.tile([NP, RS * P], F32, tag="dbg2")
                        nc.vector.tensor_copy(t2[:], cnt[:])
                        nc.sync.dma_start(out=cdbg_d.ap(), in_=t2[:])
                        t3 = dbgp.tile([NP, RS, P], F32, tag="dbg3")
                        nc.vector.tensor_copy(t3[:], ind[:])
                        nc.sync.dma_start(out=idbg_d.ap(), in_=t3[:])
                    if DBG and j == 1 and a == 1:
                        t4 = dbgp.tile([1, RS * P], F32, tag="dbg4")
                        nc.vector.tensor_copy(t4[:], pv[:])
                        nc.sync.dma_start(out=vdbg_d.ap(), in_=t4[:])
                    if a == 1:
                        svj = p2.tile([1, NPC], F32, tag=f"sv{j % 2}",
                                      name=f"sv{j}")
                        nc.scalar.activation(svj[:], pv[:], act.Copy)
                        nc.sync.dma_start(
                            out=vst[j * RS:(j + 1) * RS, :],
                            in_=svj[:].rearrange("o (s p) -> o s p", s=RS))

            if DBG:
                nc.sync.dma_start(out=wdbg_d.ap(), in_=vst[:])
            # PE-transpose the collected val rows back to pair-layout
            valT = psa.tile([NP, NPC], F32, tag="cnt", name="valT")
            for c in range(NCH):
                nc.tensor.transpose(valT[:, c * S:(c + 1) * S],
                                    vst[:, c * NP:(c + 1) * NP],
                                    ident_t[0:S, 0:S])

            resall = ph1.tile([NP, NCH, S], F32, tag="resall")
            whoall = ph1.tile([NP, NCH * S], mybir.dt.int32, tag="whoall")
            nc.vector.tensor_scalar(out=whoall[:], in0=valT[:, 0:NCH * S],
                                    scalar1=0.0, scalar2=None, op0=alu.is_gt)
            for c in range(NCH):
                d = ch[c]
                invb = d["invb"]
                val = valT[:, c * S:(c + 1) * S]
                acc = ph1.tile([NP, S], F32, tag="acc", name=f"acc{c}")
                nc.vector.tensor_scalar(out=acc[:], in0=val, scalar1=invb[:],
                                        scalar2=None, op0=alu.mult)
                lastx = ph1.tile([NP, 1], F32, tag="lastx", name=f"lastx{c}")
                nc.vector.tensor_scalar(out=lastx[:], in0=d["lastraw"],
                                        scalar1=invb[:], scalar2=None,
                                        op0=alu.mult)
                fb = ph1.tile([NP, 1], F32, tag="fb", name=f"fb{c}")
                nc.vector.tensor_tensor(out=fb[:], in0=lastx[:], in1=d["dtb"],
                                        op=alu.max)
                nc.scalar.activation(resall[:, c, :],
                                     fb[:].to_broadcast((NP, S)), act.Copy)
                nc.vector.copy_predicated(resall[:, c, :],
                                          whoall[:, c * S:(c + 1) * S], acc[:])
            res2_t = ph1.tile([NP, NCH, S], F32, tag="res2")
            nc.vector.tensor_scalar(out=res2_t[:], in0=resall[:], scalar1=1e5,
                                    scalar2=None, op0=alu.min)
            nc.sync.dma_start(
                out=res_d.ap().rearrange("(c p) s -> p c s", p=NP),
                in_=res2_t[:])

    nc.finalize()
    return nc


def _prep_inputs(time_seq, time_delta_seq, event_seq, dtime_boundary, exp_raw,
                 unif_numbers, mu, alpha, beta, type_emb):
    f = np.float32
    tds = np.asarray(time_delta_seq, f).reshape(B * L)
    dtb = np.asarray(dtime_boundary, f).reshape(B * L)
    raw0 = np.asarray(exp_raw, f).reshape(B * L, E)
    u = np.asarray(unif_numbers, f).reshape(B * L, S, E)
    ev = np.asarray(event_seq)
    mu = np.asarray(mu, f)
    alpha = np.asarray(alpha, f)
    beta = np.asarray(beta, f)
    type_emb = np.asarray(type_emb, f)

    aemb = (alpha[None, :] * type_emb)[ev].reshape(B * L, M).astype(f)

    order = np.argsort(raw0, axis=-1, kind="stable")
    raws = np.take_along_axis(raw0, order, axis=-1).astype(f)
    us = np.take_along_axis(u, order[:, None, :], axis=-1).astype(f)

    tot00 = np.log1p(np.exp((aemb + mu[None, :]).astype(np.float64))).sum(-1)
    rawmax = raw0.max(-1).astype(np.float64)
    Ddom = rawmax / (1.5 * tot00)
    fourd = (4.0 / Ddom).astype(f)
    ddom = Ddom.astype(f)

    jj = np.arange(KC)
    n = KC - 1
    frac = (1.0 + np.cos(np.pi * jj / n)) / 2.0
    linfrac = np.concatenate([np.linspace(0.0, 1.0, S0), frac]).astype(f)

    Wm = np.zeros((KC, KC))
    for k in range(KC):
        wrow = np.cos(np.pi * jj * k / n)
        wrow[0] *= 0.5
        wrow[-1] *= 0.5
        wrow *= 2.0 / n
        if k == 0 or k == n:
            wrow *= 0.5
        Wm[k] = wrow
    # tot = sum_k cc_k T_k(w2/2); T_k(w2/2) as powers of w2 (cols = power)
    t2m = np.zeros((KC, KC))
    t2m[0, 0] = 1.0
    t2m[1, 1] = 0.5
    t2m[2, 0], t2m[2, 2] = -1.0, 0.5
    t2m[3, 1], t2m[3, 3] = -1.5, 0.5
    t2m[4, 0], t2m[4, 2], t2m[4, 4] = 1.0, -2.0, 0.5
    # cw2 uses cc broadcast over rows a: am_j = sum_b cc_b * t2m[b, j]
    t2m_packed = t2m.T.reshape(1, KC * KC)  # [a=j(power), b=k(cheb)] row-major

    def bf16_bytes(x):
        x = np.ascontiguousarray(np.asarray(x, np.float32))
        u32 = x.view(np.uint32)
        r = ((u32 + 0x7FFF + ((u32 >> 16) & 1)) >> 16).astype(np.uint16)
        return r

    cst = np.concatenate([
        np.tile(mu[None, :], (NP, 1)),
        np.tile(-beta[None, :], (NP, 1)),
        np.tile(linfrac[None, :], (NP, 1)),
        np.tile(Wm.reshape(1, KC * KC).astype(f), (NP, 1)),
        np.tile(t2m_packed.astype(f), (NP, 1)),
        np.eye(NP, dtype=f),
    ], axis=1).astype(f)

    stc = np.zeros((NP, NP), f)
    for k in range(NP):
        stc[k, k] = BIG
        stc[:k, k] = -BIG
    stall = np.full((NP, NP), -BIG, f)
    stone = np.ones((NP, 1), f)
    cstb = np.concatenate([bf16_bytes(stc), bf16_bytes(stall),
                           bf16_bytes(stone)], axis=1)

    pk = np.concatenate([
        aemb, tds[:, None], ddom[:, None], fourd[:, None], dtb[:, None],
        raw0[:, E - 1:E],
    ], axis=1).astype(f)

    in_maps = []
    for core in range(N_CORES):
        rs = slice(core * P, (core + 1) * P)
        uT = np.ascontiguousarray(us[rs].transpose(2, 1, 0))   # [E, S, P]
        rwt = bf16_bytes(raws[rs].T)                           # [E, P] bf16
        in_maps.append(dict(
            pk=np.ascontiguousarray(pk[rs]),
            cst=cst,
            cstb=cstb,
            rws=np.ascontiguousarray(raws[rs]),
            rwt=np.ascontiguousarray(rwt),
            ut=uT,
        ))
    return in_maps


def kernel(time_seq, time_delta_seq, event_seq, dtime_boundary, exp_raw,
           unif_numbers, mu, alpha, beta, type_emb, _trace=False):
    if "nc" not in _CACHE:
        _CACHE["nc"] = build_program()
    nc = _CACHE["nc"]

    in_maps = _prep_inputs(time_seq, time_delta_seq, event_seq, dtime_boundary,
                           exp_raw, unif_numbers, mu, alpha, beta, type_emb)

    out = run_bass_kernel_spmd(nc, in_maps, core_ids=list(range(N_CORES)),
                               trace=_trace)
    _CACHE["last_results"] = out

    res = np.concatenate([out.results[c]["res"].reshape(BPC, L, S)
                          for c in range(N_CORES)], axis=0)
    weights = np.full((B, L, S), np.float32(1.0 / S), np.float32)
    return res, weights
